# revision 1
# baseline (speedup 1.0000x reference)
"""Trainium2 Bass kernel for nn_AlarmworkRNN: 2-track tanh RNN.

Math (per reference):
  in1 = X @ W_in1.T + b_in1 ; in2 = X @ W_in2.T + b_in2   (folded into recurrence)
  for l in 0..L-1:
      z1n = tanh(in1[l] + (z1 + z2) @ W_rec1.T)
      z2n = tanh(in2[l] + z2 @ W_rec2.T)  if l even else z2
      z1, z2 = z1n, z2n
  out = tanh(z1 @ W_out.T + b_out)       (computed on host, O=1)

Strategy: data-parallel over batch (8 cores x 64 rows). The recurrence state is
held transposed (z12T, z2T: [H=1024 -> 8 k-tiles of 128, B=64]) and used as the
matmul stationary; host-pretransposed weights are the moving operand, resident
in SBUF. The input projection X[l] @ W_in.T is folded into the same PSUM
accumulation as 2 extra k-tiles (stationary = host-pretransposed X[l].T).

Default mode "h16" (fp16, H-SPLIT column tiling): both PE column groups
accumulate ALL 10 k-tiles, but group g streams only the g-th 512-wide half of
the weight columns. The two groups' outputs are disjoint H halves on disjoint
PSUM partition ranges (rows 0:64 = H[0:512], rows 64:128 = H[512:1024]), so
no cross-group reduction is needed: per step just 2 ACT tanh ops, 8 PE
transposes (z1n -> z1nT), and 4 chunked DVE adds (z1nT + z2T -> z12T, chunked
so the next step's first matmuls start as soon as their k-tiles are ready).
The z2 track (updates on even steps only) is computed one step early; its
matmuls/transposes are interleaved as PE fill inside the z1 chain's
tanh/transpose/add latency windows. Measured ~2.6 ms for the full L=512
recurrence on 8 cores (vs 4.8-5.0 ms for the f32r K-split baseline).

HW notes (micro-benchmarked on trn2): col-tiled fp16 FD=512 matmuls stream at
~123 ns/instr (two concurrent 1 elem/cycle group streams; full-width M=128
matmuls are 214 ns -- col tiling IS the 2x); PE transposes ~38 ns batched but
~140 ns when breaking the MM stream; ACT activation [64,512] is ~750 ns
(dtype-independent); DVE adds reading PSUM are ~2.5x slower than SBUF-only
(342 vs 134 ns at FD=128). bfloat16 matmuls are ~12% faster than fp16 but the
full kernel regresses ~18% (slower ACT/DVE path) and error rises to ~1.1e-2.

Other modes kept for reference: "f16" (K-split col tiling, needs an ACT copy +
fp32 DVE add to merge the two half-K partials), "f32r" (no col tiling, ~2x
slower), "hb16" (bf16 H-split).
"""
import numpy as np

B, L, I, H = 512, 512, 256, 1024
NC = 8
BC = B // NC          # 64 batch rows per core
KH = H // 128         # 8 hidden k-tiles
KI = I // 128         # 2 input k-tiles

MODE = "h16"          # "f32r" | "f16" | "h16" | "hb16"

_CACHE = {}


def _build_h(L_steps, with_bias, reps=1, mode="h16", ablate=None):
    """H-split column-tiled fp16 pipeline.

    Differs from the K-split f16 mode: both PE column groups accumulate ALL
    k-tiles, but group g streams only the g-th 512-wide half of the weight
    columns. The two groups' PSUM outputs are disjoint H halves on disjoint
    partition ranges (rows 0:64 = H[0:512], rows 64:128 = H[512:1024]) -- no
    ACT copy / fp32 DVE add to merge halves, and one PSUM bank per step.
    tanh is 2 ACT ops per step reading the two row ranges.
    """
    import concourse.bacc as bacc
    import concourse.tile as tile
    import concourse.mybir as mybir

    F32 = mybir.dt.float32
    DT = mybir.dt.bfloat16 if mode == "hb16" else mybir.dt.float16
    Tanh = mybir.ActivationFunctionType.Tanh
    Copy = mybir.ActivationFunctionType.Copy

    nc = bacc.Bacc("TRN2", target_bir_lowering=False)
    XT = nc.declare_dram_parameter("XT", [L_steps, I, BC], DT, isOutput=False)
    W1T = nc.declare_dram_parameter("W1T", [H, H], DT, isOutput=False)
    W2T = nc.declare_dram_parameter("W2T", [H, H], DT, isOutput=False)
    Wi1T = nc.declare_dram_parameter("Wi1T", [I, H], DT, isOutput=False)
    Wi2T = nc.declare_dram_parameter("Wi2T", [I, H], DT, isOutput=False)
    IDN = nc.declare_dram_parameter("IDN2", [128, 64], DT, isOutput=False)
    ID128 = nc.declare_dram_parameter("ID128", [128, 128], DT, isOutput=False)
    ZRO = nc.declare_dram_parameter("ZRO", [128, KH * BC], DT, isOutput=False)
    if with_bias:
        BIA = nc.declare_dram_parameter("BIA", [2, H], DT, isOutput=False)
        ONE = nc.declare_dram_parameter("ONE", [1, BC], DT, isOutput=False)
    OUT = nc.declare_dram_parameter("OUT", [BC, H], F32, isOutput=True)

    with tile.TileContext(nc) as tc:
        with tc.tile_pool(name="const", bufs=1) as cpool, \
             tc.tile_pool(name="xt", bufs=6) as xpool, \
             tc.tile_pool(name="st", bufs=4) as spool, \
             tc.tile_pool(name="actt", bufs=3) as apool, \
             tc.tile_pool(name="fin", bufs=1) as fpool, \
             tc.tile_pool(name="ps1", bufs=2, space="PSUM") as ps1pool, \
             tc.tile_pool(name="ps2", bufs=2, space="PSUM") as ps2pool, \
             tc.tile_pool(name="pst", bufs=(1 if ablate == "notr" else 3), space="PSUM") as pstpool:

            # ---- resident weights: [128, ktile*H] with ktile-major free layout
            w1t_sb = cpool.tile([128, KH * H], DT)
            w2t_sb = cpool.tile([128, KH * H], DT)
            wi1t_sb = cpool.tile([128, KI * H], DT)
            wi2t_sb = cpool.tile([128, KI * H], DT)
            id_sb = cpool.tile([128, 64], DT)
            id128_sb = cpool.tile([128, 128], DT)
            nc.sync.dma_start(id_sb[:], IDN[:])
            nc.sync.dma_start(id128_sb[:], ID128[:])
            for k in range(KH):
                nc.sync.dma_start(w1t_sb[:, k*H:(k+1)*H], W1T[k*128:(k+1)*128, :])
                nc.sync.dma_start(w2t_sb[:, k*H:(k+1)*H], W2T[k*128:(k+1)*128, :])
            for k in range(KI):
                nc.sync.dma_start(wi1t_sb[:, k*H:(k+1)*H], Wi1T[k*128:(k+1)*128, :])
                nc.sync.dma_start(wi2t_sb[:, k*H:(k+1)*H], Wi2T[k*128:(k+1)*128, :])
            if with_bias:
                bia1_sb = cpool.tile([1, H], DT)
                bia2_sb = cpool.tile([1, H], DT)
                one_sb = cpool.tile([1, BC], DT)
                nc.sync.dma_start(bia1_sb[:], BIA[0:1, :])
                nc.sync.dma_start(bia2_sb[:], BIA[1:2, :])
                nc.sync.dma_start(one_sb[:], ONE[:])

            xts = {}

            def fetch_xt(l):
                if l >= L_steps:
                    return
                t = xpool.tile([128, KI * BC], DT, tag="xt")
                for k in range(KI):
                    nc.sync.dma_start(t[:, k*BC:(k+1)*BC], XT[l, k*128:(k+1)*128, :])
                xts[l] = t

            def emit_x(ps, xt_t, wi_sb, bias_row):
                """Open both groups' accumulations with the X (+bias) entries."""
                for i in range(KI):
                    for g in range(2):
                        nc.tensor.matmul(
                            ps[g*BC:(g+1)*BC, :],
                            xt_t[:, i*BC:(i+1)*BC],
                            wi_sb[:, i*H + g*512 : i*H + g*512 + 512],
                            start=(i == 0), stop=False,
                            tile_position=(0, g*64))
                if with_bias:
                    bia_sb = bia1_sb if bias_row == 0 else bia2_sb
                    for g in range(2):
                        nc.tensor.matmul(
                            ps[g*BC:(g+1)*BC, :],
                            one_sb[0:1, :],
                            bia_sb[0:1, g*512:(g+1)*512],
                            start=False, stop=False,
                            tile_position=(0, g*64))

            def emit_z(ps, zT, w_sb, k_lo, k_hi):
                """State entries k_lo..k_hi-1 for both groups; stop on k==KH-1."""
                for k in range(k_lo, k_hi):
                    for g in range(2):
                        nc.tensor.matmul(
                            ps[g*BC:(g+1)*BC, :],
                            zT[:, k*BC:(k+1)*BC],
                            w_sb[:, k*H + g*512 : k*H + g*512 + 512],
                            start=False, stop=(k == KH - 1),
                            tile_position=(0, g*64))

            def tanh_g(ps, dst, g, dt_hint=None):
                nc.scalar.activation(dst[:, g*512:(g+1)*512],
                                     ps[g*BC:(g+1)*BC, :], Tanh)

            def transposes(src, pst, k_lo, k_hi):
                """PE stationary reads must be partition-base-0: src is the
                [64, 1024] activation tile, k-tile k at cols 128*k."""
                for k in range(k_lo, k_hi):
                    nc.tensor.transpose(pst[:, k*BC:(k+1)*BC],
                                        src[:, k*128:(k+1)*128], id_sb[0:64, :])

            def inject(pst, zT):
                """pst = zT via identity matmul (out = I128.T @ zT), opening
                an accumulation the transposes then add z1nT onto: replaces
                the 4 PSUM-sourced DVE adds with 2 plain copies."""
                nc.tensor.matmul(pst[:, :], id128_sb[:, :], zT[:, :],
                                 start=True, stop=False, skip_group_check=True)

            def transposes_acc(src, pst, k_lo, k_hi):
                for k in range(k_lo, k_hi):
                    nc.tensor.matmul(pst[:, k*BC:(k+1)*BC],
                                     src[:, k*128:(k+1)*128], id_sb[0:64, :],
                                     is_transpose=True, start=False,
                                     stop=(k == KH - 1), skip_group_check=True)

            def z2_dma_transposes(z2n2, dst):
                """z2nT via the DMA xbar from the stacked [128, 512] tanh
                output (k-tile k at rows 64*(k//4), cols 128*(k%4)): DMA can
                read any partition base, and the z2 track has ~1.5 steps of
                slack, so the latency is free and PE/ACT/DVE all shed work."""
                for k in range(KH):
                    r, c = 64 * (k // 4), 128 * (k % 4)
                    nc.sync.dma_start_transpose(dst[:, k*BC:(k+1)*BC],
                                                z2n2[r:r+64, c:c+128])

            def z2_epilogue(z2n):
                """Transposes + copy for a z2n computed at the prior odd step.

                Emitted at the FOLLOWING even step, right after that step's
                critical z1 matmuls: the 8 transposes then execute during the
                z1-tanh ACT window (their z2n dependency is long satisfied),
                acting as PE fill instead of stalling the next step."""
                pst2 = pstpool.tile([128, KH * BC], DT, tag="pst")
                transposes(z2n, pst2, 0, KH)
                z2T_new = spool.tile([128, KH * BC], DT, tag="z2Tp")
                nc.vector.tensor_copy(z2T_new[:], pst2[:])
                return z2T_new

            def body():
                nonlocal xts
                xts = {}
                z12T = spool.tile([128, KH * BC], DT, tag="z12T")
                z2T = spool.tile([128, KH * BC], DT, tag="z2T")
                nc.sync.dma_start(z12T[:], ZRO[:])
                nc.sync.dma_start(z2T[:], ZRO[:])
                for l in range(min(3, L_steps)):
                    fetch_xt(l)
                # prologue: step-0 z1 opened with X; step-0 z2 fully emitted
                ps1 = ps1pool.tile([2*BC, 512], F32, tag="ps1")
                emit_x(ps1, xts[0], wi1t_sb, 0)
                ps2 = ps2pool.tile([2*BC, 512], F32, tag="ps2")
                emit_x(ps2, xts[0], wi2t_sb, 1)
                emit_z(ps2, z2T, w2t_sb, 0, KH)
                z2n = apool.tile([BC, H], DT, tag="z2n")
                tanh_g(ps2, z2n, 0)
                tanh_g(ps2, z2n, 1)
                z2T_pending = z2_epilogue(z2n)
                ps2 = None
                z2_open = False
                z2n_deferred = None
                z1n_const = None         # timing-ablation stand-ins
                if ablate in ("notanh", "notr"):
                    z1n_const = apool.tile([BC, H], DT, tag="z1n")
                    tanh_g(ps2, z1n_const, 0)
                    tanh_g(ps2, z1n_const, 1)
                if ablate == "notr":
                    pst_const = pstpool.tile([128, KH * BC], DT, tag="pstc")
                    transposes(z1n_const, pst_const, 0, KH)

                z1n_final = None
                z1n_final = None
                for l in range(L_steps):
                    even = (l % 2 == 0)
                    last = (l == L_steps - 1)
                    fetch_xt(l + 3)

                    # this step's z1 matmuls (state entering step l)
                    emit_z(ps1, z12T, w1t_sb, 0, KH)

                    if last:
                        z1n_f32 = fpool.tile([BC, H], F32)
                        tanh_g(ps1, z1n_f32, 0)
                        tanh_g(ps1, z1n_f32, 1)
                        z1n_final = z1n_f32
                        break

                    if ablate == "nopost":
                        # PE stream only: next accumulation, constant state
                        ps1 = ps1pool.tile([2*BC, 512], F32, tag="ps1")
                        emit_x(ps1, xts[l + 1], wi1t_sb, 0)
                        if (not even) and z2_open:
                            emit_z(ps2, z2T, w2t_sb, 0, KH)
                            ps2 = None
                            z2_open = False
                        if even and l + 2 < L_steps:
                            ps2 = ps2pool.tile([2*BC, 512], F32, tag="ps2")
                            emit_x(ps2, xts[l + 2], wi2t_sb, 1)
                            z2_open = True
                        if l >= 1:
                            xts.pop(l - 1, None)
                        continue

                    # tanh of this step's z1. g0 is the critical producer
                    # (transposes k0-3 wait on it), so it is split into two
                    # FD=256 ACT ops (~400ns each vs ~750ns for FD=512): the
                    # first transposes and state-add chunks start earlier.
                    if ablate != "notanh":
                        z1n = apool.tile([BC, H], DT, tag="z1n")
                        nc.scalar.activation(z1n[:, 0:256],
                                             ps1[0:BC, 0:256], Tanh)
                        nc.scalar.activation(z1n[:, 256:512],
                                             ps1[0:BC, 256:512], Tanh)
                        tanh_g(ps1, z1n, 1)
                        z1n_cur = z1n
                    else:
                        z1n_cur = z1n_const

                    # --- PE fill during the tanh-g0 ACT window ---
                    if even:
                        if z2n_deferred is not None:
                            # deferred z2 epilogue from the prior odd step
                            z2T_pending = z2_epilogue(z2n_deferred)
                            z2n_deferred = None
                        # z2 state after step l
                        if ablate != "norec":
                            z2T = z2T_pending
                    elif z2_open:
                        # first chunk of the z2 group for step l+1
                        emit_z(ps2, z2T, w2t_sb, 0, 3)

                    if even and l + 2 < L_steps:
                        # open the z2 group for step l+2 here: extra MM fill
                        # for the tanh window (its stationary is X only)
                        ps2 = ps2pool.tile([2*BC, 512], F32, tag="ps2")
                        emit_x(ps2, xts[l + 2], wi2t_sb, 1)
                        z2_open = True

                    # transposes chase the tanh chunks: k0-1 after the first
                    # FD=256 op, k2-3 after the second, k4-7 after g1
                    pst1 = pstpool.tile([128, KH * BC], DT, tag="pst")
                    transposes(z1n_cur, pst1, 0, 2)
                    transposes(z1n_cur, pst1, 2, 4)

                    ps1 = ps1pool.tile([2*BC, 512], F32, tag="ps1")
                    emit_x(ps1, xts[l + 1], wi1t_sb, 0)

                    # transposes k4..7 (wait on tanh g1)
                    transposes(z1n_cur, pst1, 4, KH)

                    # z12T = z1nT + z2T (chunked so next-step MMs start early)
                    z12T_new = spool.tile([128, KH * BC], DT, tag="z12T")
                    if ablate == "norec":
                        # break the DVE->MM edge: state stays the zero tile
                        trash = z12T_new
                        z12T_new = z12T
                    add_dst = trash if ablate == "norec" else z12T_new
                    for c in range(4):
                        nc.vector.tensor_add(add_dst[:, c*128:(c+1)*128],
                                             pst1[:, c*128:(c+1)*128],
                                             z2T[:, c*128:(c+1)*128])

                    # --- remaining fills, executing during the add latency ---
                    if (not even) and z2_open:
                        emit_z(ps2, z2T, w2t_sb, 3, KH)
                        if ablate not in ("notanh", "notr"):
                            z2n = apool.tile([BC, H], DT, tag="z2n")
                            tanh_g(ps2, z2n, 0)
                            tanh_g(ps2, z2n, 1)
                            z2n_deferred = z2n  # transposes+copy at step l+1
                        ps2 = None
                        z2_open = False

                    z12T = z12T_new
                    if l >= 1:
                        xts.pop(l - 1, None)

                return z1n_final

            if reps > 1:
                with tc.For_i(0, reps, 1):
                    z1n_final = body()
            else:
                z1n_final = body()
            nc.sync.dma_start(OUT[:], z1n_final[:])
    nc.compile()
    return nc


def _build(L_steps, with_bias, reps=1, mode=MODE):
    import concourse.bacc as bacc
    import concourse.tile as tile
    import concourse.mybir as mybir

    F32 = mybir.dt.float32
    DT = mybir.dt.float16 if mode == "f16" else mybir.dt.float32r
    col = (mode == "f16")
    Tanh = mybir.ActivationFunctionType.Tanh
    Copy = mybir.ActivationFunctionType.Copy

    nc = bacc.Bacc("TRN2", target_bir_lowering=False)
    XT = nc.declare_dram_parameter("XT", [L_steps, I, BC], DT, isOutput=False)
    W1T = nc.declare_dram_parameter("W1T", [H, H], DT, isOutput=False)
    W2T = nc.declare_dram_parameter("W2T", [H, H], DT, isOutput=False)
    Wi1T = nc.declare_dram_parameter("Wi1T", [I, H], DT, isOutput=False)
    Wi2T = nc.declare_dram_parameter("Wi2T", [I, H], DT, isOutput=False)
    IDN = nc.declare_dram_parameter("IDN", [64, 64], DT, isOutput=False)
    ZRO = nc.declare_dram_parameter("ZRO", [128, KH * BC], DT, isOutput=False)
    if with_bias:
        BIA = nc.declare_dram_parameter("BIA", [2, H], DT, isOutput=False)
        ONE = nc.declare_dram_parameter("ONE", [1, BC], DT, isOutput=False)
    OUT = nc.declare_dram_parameter("OUT", [BC, H], F32, isOutput=True)

    with tile.TileContext(nc) as tc:
        with tc.tile_pool(name="const", bufs=1) as cpool, \
             tc.tile_pool(name="xt", bufs=6) as xpool, \
             tc.tile_pool(name="st", bufs=3) as spool, \
             tc.tile_pool(name="actt", bufs=3) as apool, \
             tc.tile_pool(name="sums", bufs=3) as supool, \
             tc.tile_pool(name="fin", bufs=1) as fpool, \
             tc.tile_pool(name="ps1", bufs=2, space="PSUM") as ps1pool, \
             tc.tile_pool(name="ps2", bufs=1, space="PSUM") as ps2pool, \
             tc.tile_pool(name="pst", bufs=2, space="PSUM") as pstpool:

            # ---- resident weights: [128, ktile*H] with ktile-major free layout
            w1t_sb = cpool.tile([128, KH * H], DT)
            w2t_sb = cpool.tile([128, KH * H], DT)
            wi1t_sb = cpool.tile([128, KI * H], DT)
            wi2t_sb = cpool.tile([128, KI * H], DT)
            id_sb = cpool.tile([64, 64], DT)
            nc.sync.dma_start(id_sb[:], IDN[:])
            for k in range(KH):
                nc.sync.dma_start(w1t_sb[:, k*H:(k+1)*H], W1T[k*128:(k+1)*128, :])
                nc.sync.dma_start(w2t_sb[:, k*H:(k+1)*H], W2T[k*128:(k+1)*128, :])
            for k in range(KI):
                nc.sync.dma_start(wi1t_sb[:, k*H:(k+1)*H], Wi1T[k*128:(k+1)*128, :])
                nc.sync.dma_start(wi2t_sb[:, k*H:(k+1)*H], Wi2T[k*128:(k+1)*128, :])
            if with_bias:
                bia1_sb = cpool.tile([1, H], DT)
                bia2_sb = cpool.tile([1, H], DT)
                one_sb = cpool.tile([1, BC], DT)
                nc.sync.dma_start(bia1_sb[:], BIA[0:1, :])
                nc.sync.dma_start(bia2_sb[:], BIA[1:2, :])
                nc.sync.dma_start(one_sb[:], ONE[:])

            # ---- XT prefetch
            xts = {}

            def fetch_xt(l):
                if l >= L_steps:
                    return
                t = xpool.tile([128, KI * BC], DT, tag="xt")
                for k in range(KI):
                    nc.sync.dma_start(t[:, k*BC:(k+1)*BC], XT[l, k*128:(k+1)*128, :])
                xts[l] = t

            def groups_for(xt_t, zT, wi_sb, w_sb, bias_row):
                """Per-column-group entry lists: (stationary AP, [bank0, bank1] moving APs)."""
                def xe(k):
                    return (xt_t[:, k*BC:(k+1)*BC],
                            [wi_sb[:, k*H + b*512 : k*H + b*512 + 512] for b in range(2)])
                def ze(k):
                    return (zT[:, k*BC:(k+1)*BC],
                            [w_sb[:, k*H + b*512 : k*H + b*512 + 512] for b in range(2)])
                be = []
                if with_bias:
                    bia_sb = bia1_sb if bias_row == 0 else bia2_sb
                    be = [(one_sb[0:1, :],
                           [bia_sb[0:1, b*512:(b+1)*512] for b in range(2)])]
                if col:
                    return [[xe(0)] + be + [ze(k) for k in range(4)],
                            [xe(1)] + [ze(k) for k in range(4, KH)]]
                return [[xe(0), xe(1)] + be + [ze(k) for k in range(KH)]]

            def emit_mm(ps, groups, i_lo, i_hi):
                """Emit entries [i_lo, i_hi) of each group; start/stop per (group, bank)."""
                for g, entries in enumerate(groups):
                    n = len(entries)
                    tp = (0, g * 64) if col else None
                    rows = ps[g*BC:(g+1)*BC, :] if col else ps[0:BC, :]
                    for i in range(i_lo, min(i_hi, n)):
                        stat, movs = entries[i]
                        for b in range(2):
                            nc.tensor.matmul(
                                rows[:, b*512:(b+1)*512], stat, movs[b],
                                start=(i == 0), stop=(i == n - 1),
                                tile_position=tp)

            # "open" part = the state-independent X entries of the next step's
            # group, emitted early as PE fill for the tanh/add wait (A/B
            # measured: 2 vs 1 saves ~0.4ms total in f32r mode). Must not
            # exceed the per-group X-entry count (col mode has 1 per group),
            # else a stale-state z entry would be emitted before the update.
            N_OPEN = 1 if col else 2
            N_Z2A = 3   # entries per group of the z2 group emitted early (even tail)

            def tanh_step(ps, dst):
                """dst = tanh(pre-activation) for a whole step.

                col mode: the two column-group halves live on different PSUM
                partitions and DVE may read only one PSUM operand, so ACT
                evacuates the high half to SBUF, DVE adds, ACT applies tanh
                (chunked so the three engines pipeline)."""
                if col:
                    bsb = supool.tile([BC, H], F32, tag="bs")
                    s = supool.tile([BC, H], F32, tag="s")
                    for c in range(2):
                        nc.scalar.activation(bsb[:, c*512:(c+1)*512],
                                             ps[BC:2*BC, c*512:(c+1)*512], Copy)
                    for c in range(2):
                        nc.vector.tensor_add(s[:, c*512:(c+1)*512],
                                             ps[0:BC, c*512:(c+1)*512],
                                             bsb[:, c*512:(c+1)*512])
                    for c in range(2):
                        nc.scalar.activation(dst[:, c*512:(c+1)*512],
                                             s[:, c*512:(c+1)*512], Tanh)
                else:
                    for c in range(2):
                        nc.scalar.activation(dst[:, c*512:(c+1)*512],
                                             ps[0:BC, c*512:(c+1)*512], Tanh)

            def z2_post(ps2):
                """tanh + transposes + copy -> new pending z2T tile."""
                z2n = apool.tile([BC, H], DT, tag="z2n")
                tanh_step(ps2, z2n)
                pst2 = pstpool.tile([128, KH * BC], DT, tag="pst")
                for k in range(KH):
                    nc.tensor.transpose(pst2[:, k*BC:(k+1)*BC], z2n[:, k*128:(k+1)*128], id_sb[:])
                z2T_new = spool.tile([128, KH * BC], DT, tag="z2T")
                nc.scalar.activation(z2T_new[:], pst2[:], Copy)
                return z2T_new

            def body():
                nonlocal xts
                xts = {}
                # initial state (zeros, DMA'd so the tiles are typed producers)
                z12T = spool.tile([128, KH * BC], DT, tag="z12T")
                z2T = spool.tile([128, KH * BC], DT, tag="z2T")
                nc.sync.dma_start(z12T[:], ZRO[:])
                nc.sync.dma_start(z2T[:], ZRO[:])
                # prologue: prime XT, open step-0 z1 group, full step-0 z2 group
                for l in range(min(3, L_steps)):
                    fetch_xt(l)
                ps1 = ps1pool.tile([2*BC, H], F32, tag="ps1")
                g1 = groups_for(xts[0], z12T, wi1t_sb, w1t_sb, 0)
                emit_mm(ps1, g1, 0, N_OPEN)
                ps2 = ps2pool.tile([2*BC, H], F32, tag="ps2")
                g2 = groups_for(xts[0], z2T, wi2t_sb, w2t_sb, 1)
                emit_mm(ps2, g2, 0, 99)
                z2T_pending = z2_post(ps2)
                ps2 = g2 = None

                z1n_final = None
                for l in range(L_steps):
                    even = (l % 2 == 0)
                    last = (l == L_steps - 1)
                    fetch_xt(l + 3)

                    # close this step's z1 accumulation (state entering step l)
                    emit_mm(ps1, g1, N_OPEN, 99)

                    # z2 state after step l: updated on even steps
                    if even:
                        z2T = z2T_pending

                    # finish the z2 matmul group for step l+1 (PE fill before tanh wait)
                    if (not last) and (l + 1) % 2 == 0 and ps2 is not None:
                        emit_mm(ps2, g2, N_Z2A, 99)

                    # tanh of this step's z1 (ahead of any z2 ACT work)
                    if last:
                        z1n_f32 = fpool.tile([BC, H], F32)
                        tanh_step(ps1, z1n_f32)
                        z1n_final = z1n_f32
                        break
                    z1n = apool.tile([BC, H], DT, tag="z1n")
                    tanh_step(ps1, z1n)

                    # open next step's z1 group (independent fill before the transposes)
                    ps1 = ps1pool.tile([2*BC, H], F32, tag="ps1")
                    g1 = groups_for(xts[l + 1], z12T, wi1t_sb, w1t_sb, 0)
                    # note: g1 references z12T of step l-1 here only for the X part;
                    # the z entries are re-created below after z12T is updated.
                    emit_mm(ps1, g1, 0, N_OPEN)

                    # transpose z1n
                    pst1 = pstpool.tile([128, KH * BC], DT, tag="pst")
                    for k in range(KH):
                        nc.tensor.transpose(pst1[:, k*BC:(k+1)*BC], z1n[:, k*128:(k+1)*128], id_sb[:])

                    # z2 epilogue for step l+1 (tanh_z2 queues behind tanh_z1 on ACT;
                    # its transposes fill the PE while DVE does the add below)
                    if (not last) and (l + 1) % 2 == 0 and ps2 is not None:
                        z2T_pending = z2_post(ps2)
                        ps2 = g2 = None

                    # z12T = z1nT + z2T(after this step)
                    z12T = spool.tile([128, KH * BC], DT, tag="z12T")
                    for c in range(2):
                        nc.vector.tensor_add(z12T[:, c*256:(c+1)*256], pst1[:, c*256:(c+1)*256], z2T[:, c*256:(c+1)*256])
                    g1 = groups_for(xts[l + 1], z12T, wi1t_sb, w1t_sb, 0)

                    # open the z2 group for step l+2 at the even-step tail
                    # (fills the PE while the add completes)
                    if even and l + 2 < L_steps:
                        ps2 = ps2pool.tile([2*BC, H], F32, tag="ps2")
                        g2 = groups_for(xts[l + 2], z2T, wi2t_sb, w2t_sb, 1)
                        emit_mm(ps2, g2, 0, N_Z2A)

                    if l >= 1:
                        xts.pop(l - 1, None)

                return z1n_final

            if reps > 1:
                with tc.For_i(0, reps, 1):
                    z1n_final = body()
            else:
                z1n_final = body()
            nc.sync.dma_start(OUT[:], z1n_final[:])
    nc.compile()
    return nc


def _get_nc(L_steps, with_bias, reps=1, mode=MODE):
    key = (L_steps, with_bias, reps, mode)
    if key not in _CACHE:
        if isinstance(mode, tuple):
            _CACHE[key] = _build_h(L_steps, with_bias, reps, mode[0], mode[1])
        elif mode in ("h16", "hb16"):
            _CACHE[key] = _build_h(L_steps, with_bias, reps, mode)
        else:
            _CACHE[key] = _build(L_steps, with_bias, reps, mode)
    return _CACHE[key]


def _np_dt(mode):
    if mode == "hb16":
        import ml_dtypes
        return ml_dtypes.bfloat16
    return np.float16 if mode in ("f16", "h16") else np.float32


def _prep_in_maps(X, W_in1, b_in1, W_rec1, W_in2, b_in2, W_rec2, L_steps, mode=MODE):
    dt = _np_dt(mode)
    with_bias = bool(np.any(b_in1) or np.any(b_in2))
    w1t = np.ascontiguousarray(W_rec1.T.astype(dt))
    w2t = np.ascontiguousarray(W_rec2.T.astype(dt))
    wi1t = np.ascontiguousarray(W_in1.T.astype(dt))
    wi2t = np.ascontiguousarray(W_in2.T.astype(dt))
    if mode in ("h16", "hb16"):
        idn_key = "IDN2"
        idn = np.ascontiguousarray(np.vstack([np.eye(64, dtype=dt)] * 2))
    else:
        idn_key = "IDN"
        idn = np.eye(64, dtype=dt)
    zro = np.zeros((128, KH * BC), dt)
    in_maps = []
    for c in range(NC):
        xt = np.ascontiguousarray(
            X[c*BC:(c+1)*BC, :L_steps, :].transpose(1, 2, 0).astype(dt))
        m = {"XT": xt, "W1T": w1t, "W2T": w2t, "Wi1T": wi1t, "Wi2T": wi2t,
             idn_key: idn, "ZRO": zro}
        if mode in ("h16", "hb16"):
            m["ID128"] = np.eye(128, dtype=dt)
        if with_bias:
            m["BIA"] = np.ascontiguousarray(
                np.stack([b_in1[:, 0], b_in2[:, 0]]).astype(dt))
            m["ONE"] = np.ones((1, BC), dt)
        in_maps.append(m)
    return in_maps, with_bias


def run_device(X, W_in1, b_in1, W_rec1, W_in2, b_in2, W_rec2, L_steps=L, mode=MODE):
    """Run the recurrence on 8 cores; returns z1_final (B, H) float32."""
    from concourse.bass_utils import run_bass_kernel_spmd
    in_maps, with_bias = _prep_in_maps(X, W_in1, b_in1, W_rec1, W_in2, b_in2,
                                       W_rec2, L_steps, mode)
    nc = _get_nc(L_steps, with_bias, 1, mode)
    res = run_bass_kernel_spmd(nc, in_maps, list(range(NC)))
    return np.concatenate([res.results[c]["OUT"] for c in range(NC)], axis=0)


def kernel(X, W_in1, b_in1, W_rec1, W_in2, b_in2, W_rec2, W_out, b_out):
    X = np.asarray(X); W_out = np.asarray(W_out); b_out = np.asarray(b_out)
    assert X.shape == (B, L, I), f"unexpected X shape {X.shape}"
    z1 = run_device(X, np.asarray(W_in1), np.asarray(b_in1),
                    np.asarray(W_rec1), np.asarray(W_in2), np.asarray(b_in2),
                    np.asarray(W_rec2))
    out = np.tanh(z1.astype(np.float64) @ W_out.astype(np.float64).T
                  + b_out.astype(np.float64)[:, 0])
    return out.reshape(B, 1).astype(np.float32)



# revision 26
# speedup vs baseline: 2.2733x; 2.2733x over previous
"""Trainium2 Bass kernel for nn_AlarmworkRNN: 2-track tanh RNN.

Math (per reference):
  in1 = X @ W_in1.T + b_in1 ; in2 = X @ W_in2.T + b_in2   (folded into recurrence)
  for l in 0..L-1:
      z1n = tanh(in1[l] + (z1 + z2) @ W_rec1.T)
      z2n = tanh(in2[l] + z2 @ W_rec2.T)  if l even else z2
      z1, z2 = z1n, z2n
  out = tanh(z1 @ W_out.T + b_out)       (computed on host, O=1)

Strategy: data-parallel over batch (8 cores x 64 rows). The recurrence state is
held transposed (z12T, z2T: [H=1024 -> 8 k-tiles of 128, B=64]) and used as the
matmul stationary; host-pretransposed weights are the moving operand, resident
in SBUF. The input projection X[l] @ W_in.T is folded into the same PSUM
accumulation as 2 extra k-tiles (stationary = host-pretransposed X[l].T).

Default mode "h16" (fp16, H-SPLIT column tiling): both PE column groups
accumulate ALL 10 k-tiles, but group g streams only the g-th 512-wide half of
the weight columns. The two groups' outputs are disjoint H halves on disjoint
PSUM partition ranges (rows 0:64 = H[0:512], rows 64:128 = H[512:1024]), so
no cross-group reduction is needed: per step just 2 ACT tanh ops, 8 PE
transposes (z1n -> z1nT), and 4 chunked DVE adds (z1nT + z2T -> z12T, chunked
so the next step's first matmuls start as soon as their k-tiles are ready).
The z2 track (updates on even steps only) is computed one step early; its
matmuls/transposes are interleaved as PE fill inside the z1 chain's
tanh/transpose/add latency windows. Measured ~2.6 ms for the full L=512
recurrence on 8 cores (vs 4.8-5.0 ms for the f32r K-split baseline).

HW notes (micro-benchmarked on trn2): col-tiled fp16 FD=512 matmuls stream at
~123 ns/instr (two concurrent 1 elem/cycle group streams; full-width M=128
matmuls are 214 ns -- col tiling IS the 2x); PE transposes ~38 ns batched but
~140 ns when breaking the MM stream; ACT activation [64,512] is ~750 ns
(dtype-independent); DVE adds reading PSUM are ~2.5x slower than SBUF-only
(342 vs 134 ns at FD=128). bfloat16 matmuls are ~12% faster than fp16 but the
full kernel regresses ~18% (slower ACT/DVE path) and error rises to ~1.1e-2.

Other modes kept for reference: "f16" (K-split col tiling, needs an ACT copy +
fp32 DVE add to merge the two half-K partials), "f32r" (no col tiling, ~2x
slower), "hb16" (bf16 H-split).
"""
import numpy as np

B, L, I, H = 512, 512, 256, 1024
NC = 8
BC = B // NC          # 64 batch rows per core
KH = H // 128         # 8 hidden k-tiles
KI = I // 128         # 2 input k-tiles

MODE = "h16"          # "f32r" | "f16" | "h16" | "hb16" | "v2"

# _build_v2 schedule knobs (read at build time; key your cache accordingly)
V2_OPTS = {
    # z2 transpose route:
    #   dma  - xbar transpose DMAs issued at the odd tail (blocks SP queue
    #          head on the z2-tanh dependency)
    #   dma2 - xbar transpose DMAs issued at the next even step (dependency
    #          already met -> no queue-head block); z2split should be 0
    #   dve/gp/act - PE transposes deferred to the even step + PSUM->SBUF
    #          copy on that engine (h16 used dve)
    "z2tr": "dve",
    "z2split": 2,      # z2 state k-tiles emitted at the even tail (0..8)
    "dmaq": "sync",    # engine queue issuing the transpose DMAs: sync|scalar
    "z2stack": 0,      # stacked z2 tanh faults on HW (NRT_EXEC_UNIT) -- keep 0
    "ilv": 1,
}

_CACHE = {}


def _build_h(L_steps, with_bias, reps=1, mode="h16", ablate=None):
    """H-split column-tiled fp16 pipeline.

    Differs from the K-split f16 mode: both PE column groups accumulate ALL
    k-tiles, but group g streams only the g-th 512-wide half of the weight
    columns. The two groups' PSUM outputs are disjoint H halves on disjoint
    partition ranges (rows 0:64 = H[0:512], rows 64:128 = H[512:1024]) -- no
    ACT copy / fp32 DVE add to merge halves, and one PSUM bank per step.
    tanh is 2 ACT ops per step reading the two row ranges.
    """
    import concourse.bacc as bacc
    import concourse.tile as tile
    import concourse.mybir as mybir

    F32 = mybir.dt.float32
    DT = mybir.dt.bfloat16 if mode == "hb16" else mybir.dt.float16
    Tanh = mybir.ActivationFunctionType.Tanh
    Copy = mybir.ActivationFunctionType.Copy

    nc = bacc.Bacc("TRN2", target_bir_lowering=False)
    XT = nc.declare_dram_parameter("XT", [L_steps, I, BC], DT, isOutput=False)
    W1T = nc.declare_dram_parameter("W1T", [H, H], DT, isOutput=False)
    W2T = nc.declare_dram_parameter("W2T", [H, H], DT, isOutput=False)
    Wi1T = nc.declare_dram_parameter("Wi1T", [I, H], DT, isOutput=False)
    Wi2T = nc.declare_dram_parameter("Wi2T", [I, H], DT, isOutput=False)
    IDN = nc.declare_dram_parameter("IDN2", [128, 64], DT, isOutput=False)
    ID128 = nc.declare_dram_parameter("ID128", [128, 128], DT, isOutput=False)
    ZRO = nc.declare_dram_parameter("ZRO", [128, KH * BC], DT, isOutput=False)
    if with_bias:
        BIA = nc.declare_dram_parameter("BIA", [2, H], DT, isOutput=False)
        ONE = nc.declare_dram_parameter("ONE", [1, BC], DT, isOutput=False)
    OUT = nc.declare_dram_parameter("OUT", [BC, H], F32, isOutput=True)

    with tile.TileContext(nc) as tc:
        with tc.tile_pool(name="const", bufs=1) as cpool, \
             tc.tile_pool(name="xt", bufs=6) as xpool, \
             tc.tile_pool(name="st", bufs=4) as spool, \
             tc.tile_pool(name="actt", bufs=3) as apool, \
             tc.tile_pool(name="fin", bufs=1) as fpool, \
             tc.tile_pool(name="ps1", bufs=2, space="PSUM") as ps1pool, \
             tc.tile_pool(name="ps2", bufs=2, space="PSUM") as ps2pool, \
             tc.tile_pool(name="pst", bufs=(1 if ablate == "notr" else 3), space="PSUM") as pstpool:

            # ---- resident weights: [128, ktile*H] with ktile-major free layout
            w1t_sb = cpool.tile([128, KH * H], DT)
            w2t_sb = cpool.tile([128, KH * H], DT)
            wi1t_sb = cpool.tile([128, KI * H], DT)
            wi2t_sb = cpool.tile([128, KI * H], DT)
            id_sb = cpool.tile([128, 64], DT)
            id128_sb = cpool.tile([128, 128], DT)
            nc.sync.dma_start(id_sb[:], IDN[:])
            nc.sync.dma_start(id128_sb[:], ID128[:])
            for k in range(KH):
                nc.sync.dma_start(w1t_sb[:, k*H:(k+1)*H], W1T[k*128:(k+1)*128, :])
                nc.sync.dma_start(w2t_sb[:, k*H:(k+1)*H], W2T[k*128:(k+1)*128, :])
            for k in range(KI):
                nc.sync.dma_start(wi1t_sb[:, k*H:(k+1)*H], Wi1T[k*128:(k+1)*128, :])
                nc.sync.dma_start(wi2t_sb[:, k*H:(k+1)*H], Wi2T[k*128:(k+1)*128, :])
            if with_bias:
                bia1_sb = cpool.tile([1, H], DT)
                bia2_sb = cpool.tile([1, H], DT)
                one_sb = cpool.tile([1, BC], DT)
                nc.sync.dma_start(bia1_sb[:], BIA[0:1, :])
                nc.sync.dma_start(bia2_sb[:], BIA[1:2, :])
                nc.sync.dma_start(one_sb[:], ONE[:])

            xts = {}

            def fetch_xt(l):
                if l >= L_steps:
                    return
                t = xpool.tile([128, KI * BC], DT, tag="xt")
                for k in range(KI):
                    nc.sync.dma_start(t[:, k*BC:(k+1)*BC], XT[l, k*128:(k+1)*128, :])
                xts[l] = t

            def emit_x(ps, xt_t, wi_sb, bias_row):
                """Open both groups' accumulations with the X (+bias) entries."""
                for i in range(KI):
                    for g in range(2):
                        nc.tensor.matmul(
                            ps[g*BC:(g+1)*BC, :],
                            xt_t[:, i*BC:(i+1)*BC],
                            wi_sb[:, i*H + g*512 : i*H + g*512 + 512],
                            start=(i == 0), stop=False,
                            tile_position=(0, g*64))
                if with_bias:
                    bia_sb = bia1_sb if bias_row == 0 else bia2_sb
                    for g in range(2):
                        nc.tensor.matmul(
                            ps[g*BC:(g+1)*BC, :],
                            one_sb[0:1, :],
                            bia_sb[0:1, g*512:(g+1)*512],
                            start=False, stop=False,
                            tile_position=(0, g*64))

            def emit_z(ps, zT, w_sb, k_lo, k_hi):
                """State entries k_lo..k_hi-1 for both groups; stop on k==KH-1."""
                for k in range(k_lo, k_hi):
                    for g in range(2):
                        nc.tensor.matmul(
                            ps[g*BC:(g+1)*BC, :],
                            zT[:, k*BC:(k+1)*BC],
                            w_sb[:, k*H + g*512 : k*H + g*512 + 512],
                            start=False, stop=(k == KH - 1),
                            tile_position=(0, g*64))

            def tanh_g(ps, dst, g, dt_hint=None):
                nc.scalar.activation(dst[:, g*512:(g+1)*512],
                                     ps[g*BC:(g+1)*BC, :], Tanh)

            def transposes(src, pst, k_lo, k_hi):
                """PE stationary reads must be partition-base-0: src is the
                [64, 1024] activation tile, k-tile k at cols 128*k."""
                for k in range(k_lo, k_hi):
                    nc.tensor.transpose(pst[:, k*BC:(k+1)*BC],
                                        src[:, k*128:(k+1)*128], id_sb[0:64, :])

            def inject(pst, zT):
                """pst = zT via identity matmul (out = I128.T @ zT), opening
                an accumulation the transposes then add z1nT onto: replaces
                the 4 PSUM-sourced DVE adds with 2 plain copies."""
                nc.tensor.matmul(pst[:, :], id128_sb[:, :], zT[:, :],
                                 start=True, stop=False, skip_group_check=True)

            def transposes_acc(src, pst, k_lo, k_hi):
                for k in range(k_lo, k_hi):
                    nc.tensor.matmul(pst[:, k*BC:(k+1)*BC],
                                     src[:, k*128:(k+1)*128], id_sb[0:64, :],
                                     is_transpose=True, start=False,
                                     stop=(k == KH - 1), skip_group_check=True)

            def z2_dma_transposes(z2n2, dst):
                """z2nT via the DMA xbar from the stacked [128, 512] tanh
                output (k-tile k at rows 64*(k//4), cols 128*(k%4)): DMA can
                read any partition base, and the z2 track has ~1.5 steps of
                slack, so the latency is free and PE/ACT/DVE all shed work."""
                for k in range(KH):
                    r, c = 64 * (k // 4), 128 * (k % 4)
                    nc.sync.dma_start_transpose(dst[:, k*BC:(k+1)*BC],
                                                z2n2[r:r+64, c:c+128])

            def z2_epilogue(z2n):
                """Transposes + copy for a z2n computed at the prior odd step.

                Emitted at the FOLLOWING even step, right after that step's
                critical z1 matmuls: the 8 transposes then execute during the
                z1-tanh ACT window (their z2n dependency is long satisfied),
                acting as PE fill instead of stalling the next step."""
                pst2 = pstpool.tile([128, KH * BC], DT, tag="pst")
                transposes(z2n, pst2, 0, KH)
                z2T_new = spool.tile([128, KH * BC], DT, tag="z2Tp")
                nc.vector.tensor_copy(z2T_new[:], pst2[:])
                return z2T_new

            def body():
                nonlocal xts
                xts = {}
                z12T = spool.tile([128, KH * BC], DT, tag="z12T")
                z2T = spool.tile([128, KH * BC], DT, tag="z2T")
                nc.sync.dma_start(z12T[:], ZRO[:])
                nc.sync.dma_start(z2T[:], ZRO[:])
                for l in range(min(3, L_steps)):
                    fetch_xt(l)
                # prologue: step-0 z1 opened with X; step-0 z2 fully emitted
                ps1 = ps1pool.tile([2*BC, 512], F32, tag="ps1")
                emit_x(ps1, xts[0], wi1t_sb, 0)
                ps2 = ps2pool.tile([2*BC, 512], F32, tag="ps2")
                emit_x(ps2, xts[0], wi2t_sb, 1)
                emit_z(ps2, z2T, w2t_sb, 0, KH)
                z2n = apool.tile([BC, H], DT, tag="z2n")
                tanh_g(ps2, z2n, 0)
                tanh_g(ps2, z2n, 1)
                z2T_pending = z2_epilogue(z2n)
                ps2 = None
                z2_open = False
                z2n_deferred = None
                z1n_const = None         # timing-ablation stand-ins
                if ablate in ("notanh", "notr"):
                    z1n_const = apool.tile([BC, H], DT, tag="z1n")
                    tanh_g(ps2, z1n_const, 0)
                    tanh_g(ps2, z1n_const, 1)
                if ablate == "notr":
                    pst_const = pstpool.tile([128, KH * BC], DT, tag="pstc")
                    transposes(z1n_const, pst_const, 0, KH)

                z1n_final = None
                z1n_final = None
                for l in range(L_steps):
                    even = (l % 2 == 0)
                    last = (l == L_steps - 1)
                    fetch_xt(l + 3)

                    # this step's z1 matmuls (state entering step l)
                    emit_z(ps1, z12T, w1t_sb, 0, KH)

                    if last:
                        z1n_f32 = fpool.tile([BC, H], F32)
                        tanh_g(ps1, z1n_f32, 0)
                        tanh_g(ps1, z1n_f32, 1)
                        z1n_final = z1n_f32
                        break

                    if ablate == "nopost":
                        # PE stream only: next accumulation, constant state
                        ps1 = ps1pool.tile([2*BC, 512], F32, tag="ps1")
                        emit_x(ps1, xts[l + 1], wi1t_sb, 0)
                        if (not even) and z2_open:
                            emit_z(ps2, z2T, w2t_sb, 0, KH)
                            ps2 = None
                            z2_open = False
                        if even and l + 2 < L_steps:
                            ps2 = ps2pool.tile([2*BC, 512], F32, tag="ps2")
                            emit_x(ps2, xts[l + 2], wi2t_sb, 1)
                            z2_open = True
                        if l >= 1:
                            xts.pop(l - 1, None)
                        continue

                    # tanh of this step's z1. g0 is the critical producer
                    # (transposes k0-3 wait on it), so it is split into two
                    # FD=256 ACT ops (~400ns each vs ~750ns for FD=512): the
                    # first transposes and state-add chunks start earlier.
                    if ablate != "notanh":
                        z1n = apool.tile([BC, H], DT, tag="z1n")
                        nc.scalar.activation(z1n[:, 0:256],
                                             ps1[0:BC, 0:256], Tanh)
                        nc.scalar.activation(z1n[:, 256:512],
                                             ps1[0:BC, 256:512], Tanh)
                        tanh_g(ps1, z1n, 1)
                        z1n_cur = z1n
                    else:
                        z1n_cur = z1n_const

                    # --- PE fill during the tanh-g0 ACT window ---
                    if even:
                        if z2n_deferred is not None:
                            # deferred z2 epilogue from the prior odd step
                            z2T_pending = z2_epilogue(z2n_deferred)
                            z2n_deferred = None
                        # z2 state after step l
                        if ablate != "norec":
                            z2T = z2T_pending
                    elif z2_open:
                        # first chunk of the z2 group for step l+1
                        emit_z(ps2, z2T, w2t_sb, 0, 3)

                    if even and l + 2 < L_steps:
                        # open the z2 group for step l+2 here: extra MM fill
                        # for the tanh window (its stationary is X only)
                        ps2 = ps2pool.tile([2*BC, 512], F32, tag="ps2")
                        emit_x(ps2, xts[l + 2], wi2t_sb, 1)
                        z2_open = True

                    # transposes chase the tanh chunks: k0-1 after the first
                    # FD=256 op, k2-3 after the second, k4-7 after g1
                    pst1 = pstpool.tile([128, KH * BC], DT, tag="pst")
                    transposes(z1n_cur, pst1, 0, 2)
                    transposes(z1n_cur, pst1, 2, 4)

                    ps1 = ps1pool.tile([2*BC, 512], F32, tag="ps1")
                    emit_x(ps1, xts[l + 1], wi1t_sb, 0)

                    # transposes k4..7 (wait on tanh g1)
                    transposes(z1n_cur, pst1, 4, KH)

                    # z12T = z1nT + z2T (chunked so next-step MMs start early)
                    z12T_new = spool.tile([128, KH * BC], DT, tag="z12T")
                    if ablate == "norec":
                        # break the DVE->MM edge: state stays the zero tile
                        trash = z12T_new
                        z12T_new = z12T
                    add_dst = trash if ablate == "norec" else z12T_new
                    for c in range(4):
                        nc.vector.tensor_add(add_dst[:, c*128:(c+1)*128],
                                             pst1[:, c*128:(c+1)*128],
                                             z2T[:, c*128:(c+1)*128])

                    # --- remaining fills, executing during the add latency ---
                    if (not even) and z2_open:
                        emit_z(ps2, z2T, w2t_sb, 3, KH)
                        if ablate not in ("notanh", "notr"):
                            z2n = apool.tile([BC, H], DT, tag="z2n")
                            tanh_g(ps2, z2n, 0)
                            tanh_g(ps2, z2n, 1)
                            z2n_deferred = z2n  # transposes+copy at step l+1
                        ps2 = None
                        z2_open = False

                    z12T = z12T_new
                    if l >= 1:
                        xts.pop(l - 1, None)

                return z1n_final

            if reps > 1:
                with tc.For_i(0, reps, 1):
                    z1n_final = body()
            else:
                z1n_final = body()
            nc.sync.dma_start(OUT[:], z1n_final[:])
    nc.compile()
    return nc


def _build_v2(L_steps, with_bias, reps=1, mode="v2"):
    """Balanced-parity fp16 H-split pipeline (evolution of h16).

    Two changes vs _build_h:
    1. The z2 track's transposes ride the DMA crossbar instead of the PE:
       one dma_start_transpose per tanh half, using a 3D blocked-transpose
       destination AP ([128, k, 64] <- [64, k*128]), shedding 8 PE
       transposes + a [128,512] PSUM-sourced DVE copy per step pair. DVE
       then only does the 4 z12T add chunks, so the even-step adds are
       never queued behind the copy (which previously stalled the next
       step's first matmul by ~0.8us on even steps).
    2. The z2 matmul group is split 10/10 across the step pair (x-open +
       state k0-2 at the even tail once the DMA'd z2T lands, k3-7 at the
       odd tail), so both parities carry ~30 matmul instrs of PE fill
       around the step boundary instead of 20/40.
    """
    import concourse.bacc as bacc
    import concourse.tile as tile
    import concourse.mybir as mybir

    F32 = mybir.dt.float32
    DT = mybir.dt.bfloat16 if mode == "vb2" else mybir.dt.float16
    Tanh = mybir.ActivationFunctionType.Tanh
    # v3: stacked activations -- tanh reads the full [128, .] PSUM range
    # (both column groups at once; ACT cost is free-dim-bound, partitions
    # are free) and the k4-7 transposes read their stationary at partition
    # base 64 with tile_position=(64, 0)
    stack = (mode == "v3")

    nc = bacc.Bacc("TRN2", target_bir_lowering=False)
    XT = nc.declare_dram_parameter("XT", [L_steps, I, BC], DT, isOutput=False)
    W1T = nc.declare_dram_parameter("W1T", [H, H], DT, isOutput=False)
    W2T = nc.declare_dram_parameter("W2T", [H, H], DT, isOutput=False)
    Wi1T = nc.declare_dram_parameter("Wi1T", [I, H], DT, isOutput=False)
    Wi2T = nc.declare_dram_parameter("Wi2T", [I, H], DT, isOutput=False)
    IDN = nc.declare_dram_parameter("IDN2", [128, 64], DT, isOutput=False)
    ZRO = nc.declare_dram_parameter("ZRO", [128, KH * BC], DT, isOutput=False)
    if with_bias:
        BIA = nc.declare_dram_parameter("BIA", [2, H], DT, isOutput=False)
        ONE = nc.declare_dram_parameter("ONE", [1, BC], DT, isOutput=False)
    OUT = nc.declare_dram_parameter("OUT", [BC, H], F32, isOutput=True)

    with tile.TileContext(nc) as tc:
        with tc.tile_pool(name="const", bufs=1) as cpool, \
             tc.tile_pool(name="xt", bufs=6) as xpool, \
             tc.tile_pool(name="st", bufs=6) as spool, \
             tc.tile_pool(name="actt", bufs=4) as apool, \
             tc.tile_pool(name="fin", bufs=1) as fpool, \
             tc.tile_pool(name="ps1", bufs=2, space="PSUM") as ps1pool, \
             tc.tile_pool(name="ps2", bufs=2, space="PSUM") as ps2pool, \
             tc.tile_pool(name="pst", bufs=2, space="PSUM") as pstpool:

            w1t_sb = cpool.tile([128, KH * H], DT)
            w2t_sb = cpool.tile([128, KH * H], DT)
            wi1t_sb = cpool.tile([128, KI * H], DT)
            wi2t_sb = cpool.tile([128, KI * H], DT)
            id_sb = cpool.tile([128, 64], DT)
            nc.sync.dma_start(id_sb[:], IDN[:])
            for k in range(KH):
                nc.sync.dma_start(w1t_sb[:, k*H:(k+1)*H], W1T[k*128:(k+1)*128, :])
                nc.sync.dma_start(w2t_sb[:, k*H:(k+1)*H], W2T[k*128:(k+1)*128, :])
            for k in range(KI):
                nc.sync.dma_start(wi1t_sb[:, k*H:(k+1)*H], Wi1T[k*128:(k+1)*128, :])
                nc.sync.dma_start(wi2t_sb[:, k*H:(k+1)*H], Wi2T[k*128:(k+1)*128, :])
            if with_bias:
                bia1_sb = cpool.tile([1, H], DT)
                bia2_sb = cpool.tile([1, H], DT)
                one_sb = cpool.tile([1, BC], DT)
                nc.sync.dma_start(bia1_sb[:], BIA[0:1, :])
                nc.sync.dma_start(bia2_sb[:], BIA[1:2, :])
                nc.sync.dma_start(one_sb[:], ONE[:])

            xts = {}

            def fetch_xt(l):
                if l >= L_steps:
                    return
                t = xpool.tile([128, KI * BC], DT, tag="xt")
                for k in range(KI):
                    nc.sync.dma_start(t[:, k*BC:(k+1)*BC], XT[l, k*128:(k+1)*128, :])
                xts[l] = t

            def emit_x(ps, xt_t, wi_sb, bias_row):
                for i in range(KI):
                    for g in range(2):
                        nc.tensor.matmul(
                            ps[g*BC:(g+1)*BC, :],
                            xt_t[:, i*BC:(i+1)*BC],
                            wi_sb[:, i*H + g*512 : i*H + g*512 + 512],
                            start=(i == 0), stop=False,
                            tile_position=(0, g*64))
                if with_bias:
                    bia_sb = bia1_sb if bias_row == 0 else bia2_sb
                    for g in range(2):
                        nc.tensor.matmul(
                            ps[g*BC:(g+1)*BC, :],
                            one_sb[0:1, :],
                            bia_sb[0:1, g*512:(g+1)*512],
                            start=False, stop=False,
                            tile_position=(0, g*64))

            def emit_z(ps, zT, w_sb, k_lo, k_hi):
                for k in range(k_lo, k_hi):
                    for g in range(2):
                        nc.tensor.matmul(
                            ps[g*BC:(g+1)*BC, :],
                            zT[:, k*BC:(k+1)*BC],
                            w_sb[:, k*H + g*512 : k*H + g*512 + 512],
                            start=False, stop=(k == KH - 1),
                            tile_position=(0, g*64))

            def tanh_g(ps, dst, g):
                nc.scalar.activation(dst[:, g*512:(g+1)*512],
                                     ps[g*BC:(g+1)*BC, :], Tanh)

            def transposes(src, pst, k_lo, k_hi):
                for k in range(k_lo, k_hi):
                    nc.tensor.transpose(pst[:, k*BC:(k+1)*BC],
                                        src[:, k*128:(k+1)*128], id_sb[0:64, :])

            def transposes_s(src, pst, ks):
                """Transposes from the stacked [128, 512] activation tile:
                k0-3 live on partitions 0:64, k4-7 on 64:128 (stationary at
                partition base 64, a legal 64-row tile position)."""
                for k in ks:
                    if k < 4:
                        nc.tensor.transpose(pst[:, k*BC:(k+1)*BC],
                                            src[0:64, k*128:(k+1)*128],
                                            id_sb[0:64, :])
                    else:
                        c = (k - 4) * 128
                        nc.tensor.transpose(pst[:, k*BC:(k+1)*BC],
                                            src[64:128, c:c+128],
                                            id_sb[64:128, :])

            z2tr = V2_OPTS.get("z2tr", "dma")
            z2split = V2_OPTS.get("z2split", 3)
            dmaq = nc.scalar if V2_OPTS.get("dmaq") == "scalar" else nc.sync

            def dma_tr(dst, src, k_lo, k_hi):
                """dst[:, k*BC:(k+1)*BC] = src[:, k*128:(k+1)*128].T for
                k in [k_lo, k_hi) -- one blocked-transpose DMA: the 3D
                destination AP distributes the 128-row blocks of the
                conceptual [512, 64] transpose k-major (same idiom as
                tile_matmul's kxn_tile xbar path)."""
                view = dst[:, k_lo*BC:k_hi*BC].rearrange(
                    "p (k b) -> p k b", b=BC)
                dmaq.dma_start_transpose(
                    view, src[0:BC, k_lo*128:k_hi*128])

            def pe_tr_epilogue(z2n_src):
                """h16-style: PE transposes + PSUM->SBUF copy -> new z2T."""
                pst2 = pstpool.tile([128, KH * BC], DT, tag="pst")
                if stack and V2_OPTS.get("z2stack", 1):
                    transposes_s(z2n_src, pst2, range(KH))
                else:
                    transposes(z2n_src, pst2, 0, KH)
                z2T_new = spool.tile([128, KH * BC], DT, tag="z2T")
                if z2tr == "gp":
                    nc.gpsimd.tensor_copy(z2T_new[:], pst2[:])
                elif z2tr == "act":
                    nc.scalar.activation(
                        z2T_new[:], pst2[:],
                        mybir.ActivationFunctionType.Copy)
                else:
                    nc.vector.tensor_copy(z2T_new[:], pst2[:])
                return z2T_new

            def body():
                nonlocal xts
                xts = {}
                z12T = spool.tile([128, KH * BC], DT, tag="z12T")
                z2T = spool.tile([128, KH * BC], DT, tag="z2T")
                nc.sync.dma_start(z12T[:], ZRO[:])
                nc.sync.dma_start(z2T[:], ZRO[:])
                for l in range(min(3, L_steps)):
                    fetch_xt(l)
                # prologue: step-0 z1 opened with X; step-0 z2 fully emitted,
                # tanh'd, and sent through the DMA transpose
                ps1 = ps1pool.tile([2*BC, 512], F32, tag="ps1")
                emit_x(ps1, xts[0], wi1t_sb, 0)
                ps2 = ps2pool.tile([2*BC, 512], F32, tag="ps2")
                emit_x(ps2, xts[0], wi2t_sb, 1)
                emit_z(ps2, z2T, w2t_sb, 0, KH)
                z2n_deferred = None
                z2T_pending = None
                z2stack = stack and V2_OPTS.get("z2stack", 1)
                if z2stack:
                    z2n = apool.tile([128, 512], DT, tag="z2n")
                    nc.scalar.activation(z2n[:, 0:256], ps2[0:128, 0:256], Tanh)
                    nc.scalar.activation(z2n[:, 256:512], ps2[0:128, 256:512], Tanh)
                    z2n_deferred = z2n
                elif z2tr == "dma":
                    z2n = apool.tile([BC, H], DT, tag="z2n")
                    z2T_pending = spool.tile([128, KH * BC], DT, tag="z2T")
                    tanh_g(ps2, z2n, 0)
                    dma_tr(z2T_pending, z2n, 0, 4)
                    tanh_g(ps2, z2n, 1)
                    dma_tr(z2T_pending, z2n, 4, KH)
                else:
                    z2n = apool.tile([BC, H], DT, tag="z2n")
                    tanh_g(ps2, z2n, 0)
                    tanh_g(ps2, z2n, 1)
                    z2n_deferred = z2n
                ps2 = None
                z2_open = False

                z1n_final = None
                for l in range(L_steps):
                    even = (l % 2 == 0)
                    last = (l == L_steps - 1)
                    fetch_xt(l + 3)

                    # this step's z1 state matmuls
                    emit_z(ps1, z12T, w1t_sb, 0, KH)

                    # z2 group work at the step tails (PE fill):
                    if even:
                        if l + 2 < L_steps:
                            ps2 = ps2pool.tile([2*BC, 512], F32, tag="ps2")
                            emit_x(ps2, xts[l + 2], wi2t_sb, 1)
                            z2_open = True
                        # materialize z2 state after step l as z2T
                        if z2tr == "dma":
                            z2T = z2T_pending
                        elif z2tr == "dma2":
                            z2T_new = spool.tile([128, KH * BC], DT, tag="z2T")
                            dma_tr(z2T_new, z2n_deferred, 0, 4)
                            dma_tr(z2T_new, z2n_deferred, 4, KH)
                            z2n_deferred = None
                            z2T = z2T_new
                        else:
                            z2T = pe_tr_epilogue(z2n_deferred)
                            z2n_deferred = None
                        if z2_open and z2split > 0:
                            emit_z(ps2, z2T, w2t_sb, 0, z2split)
                    elif z2_open:
                        emit_z(ps2, z2T, w2t_sb, z2split, KH)

                    if last:
                        z1n_f32 = fpool.tile([BC, H], F32)
                        tanh_g(ps1, z1n_f32, 0)
                        tanh_g(ps1, z1n_f32, 1)
                        z1n_final = z1n_f32
                        break

                    if stack:
                        # z1 tanh over the full 128-partition PSUM range in
                        # 2x FD=256 ops: op1 delivers k0-1 AND k4-5, op2 the
                        # rest -- half the ACT latency of the g-shifted form
                        z1n = apool.tile([128, 512], DT, tag="z1n")
                        nc.scalar.activation(z1n[:, 0:256],
                                             ps1[0:128, 0:256], Tanh)
                        nc.scalar.activation(z1n[:, 256:512],
                                             ps1[0:128, 256:512], Tanh)
                    else:
                        z1n = apool.tile([BC, H], DT, tag="z1n")
                        nc.scalar.activation(z1n[:, 0:256],
                                             ps1[0:BC, 0:256], Tanh)
                        nc.scalar.activation(z1n[:, 256:512],
                                             ps1[0:BC, 256:512], Tanh)
                        tanh_g(ps1, z1n, 1)

                    # z2 tanh (+ DMA transposes) -- odd steps; group closed
                    if (not even) and z2_open:
                        if z2stack:
                            z2n = apool.tile([128, 512], DT, tag="z2n")
                            nc.scalar.activation(z2n[:, 0:256], ps2[0:128, 0:256], Tanh)
                            nc.scalar.activation(z2n[:, 256:512], ps2[0:128, 256:512], Tanh)
                            z2n_deferred = z2n
                        elif z2tr == "dma":
                            z2n = apool.tile([BC, H], DT, tag="z2n")
                            z2T_new = spool.tile([128, KH * BC], DT, tag="z2T")
                            tanh_g(ps2, z2n, 0)
                            dma_tr(z2T_new, z2n, 0, 4)
                            tanh_g(ps2, z2n, 1)
                            dma_tr(z2T_new, z2n, 4, KH)
                            z2T_pending = z2T_new
                        else:
                            z2n = apool.tile([BC, H], DT, tag="z2n")
                            tanh_g(ps2, z2n, 0)
                            tanh_g(ps2, z2n, 1)
                            z2n_deferred = z2n
                        ps2 = None
                        z2_open = False

                    pst1 = pstpool.tile([128, KH * BC], DT, tag="pst")
                    z12T_new = spool.tile([128, KH * BC], DT, tag="z12T")

                    def add_c(c):
                        nc.vector.tensor_add(z12T_new[:, c*128:(c+1)*128],
                                             pst1[:, c*128:(c+1)*128],
                                             z2T[:, c*128:(c+1)*128])

                    if stack and V2_OPTS.get("ilv", 1):
                        # per-chunk: transpose pair -> its add, ordered by
                        # tanh-op readiness (op1: k0-1,k4-5; op2: k2-3,k6-7)
                        transposes_s(z1n, pst1, [0, 1])
                        add_c(0)
                        transposes_s(z1n, pst1, [4, 5])
                        add_c(2)
                        ps1 = ps1pool.tile([2*BC, 512], F32, tag="ps1")
                        emit_x(ps1, xts[l + 1], wi1t_sb, 0)
                        transposes_s(z1n, pst1, [2, 3])
                        add_c(1)
                        transposes_s(z1n, pst1, [6, 7])
                        add_c(3)
                    elif stack:
                        transposes_s(z1n, pst1, [0, 1, 2, 3])
                        ps1 = ps1pool.tile([2*BC, 512], F32, tag="ps1")
                        emit_x(ps1, xts[l + 1], wi1t_sb, 0)
                        transposes_s(z1n, pst1, [4, 5, 6, 7])
                        for c in range(4):
                            add_c(c)
                    else:
                        transposes(z1n, pst1, 0, 2)
                        transposes(z1n, pst1, 2, 4)
                        ps1 = ps1pool.tile([2*BC, 512], F32, tag="ps1")
                        emit_x(ps1, xts[l + 1], wi1t_sb, 0)
                        transposes(z1n, pst1, 4, KH)
                        for c in range(4):
                            add_c(c)
                    z12T = z12T_new
                    if l >= 1:
                        xts.pop(l - 1, None)

                return z1n_final

            if reps > 1:
                with tc.For_i(0, reps, 1):
                    z1n_final = body()
            else:
                z1n_final = body()
            nc.sync.dma_start(OUT[:], z1n_final[:])
    nc.compile()
    return nc


def _build(L_steps, with_bias, reps=1, mode=MODE):
    import concourse.bacc as bacc
    import concourse.tile as tile
    import concourse.mybir as mybir

    F32 = mybir.dt.float32
    DT = mybir.dt.float16 if mode == "f16" else mybir.dt.float32r
    col = (mode == "f16")
    Tanh = mybir.ActivationFunctionType.Tanh
    Copy = mybir.ActivationFunctionType.Copy

    nc = bacc.Bacc("TRN2", target_bir_lowering=False)
    XT = nc.declare_dram_parameter("XT", [L_steps, I, BC], DT, isOutput=False)
    W1T = nc.declare_dram_parameter("W1T", [H, H], DT, isOutput=False)
    W2T = nc.declare_dram_parameter("W2T", [H, H], DT, isOutput=False)
    Wi1T = nc.declare_dram_parameter("Wi1T", [I, H], DT, isOutput=False)
    Wi2T = nc.declare_dram_parameter("Wi2T", [I, H], DT, isOutput=False)
    IDN = nc.declare_dram_parameter("IDN", [64, 64], DT, isOutput=False)
    ZRO = nc.declare_dram_parameter("ZRO", [128, KH * BC], DT, isOutput=False)
    if with_bias:
        BIA = nc.declare_dram_parameter("BIA", [2, H], DT, isOutput=False)
        ONE = nc.declare_dram_parameter("ONE", [1, BC], DT, isOutput=False)
    OUT = nc.declare_dram_parameter("OUT", [BC, H], F32, isOutput=True)

    with tile.TileContext(nc) as tc:
        with tc.tile_pool(name="const", bufs=1) as cpool, \
             tc.tile_pool(name="xt", bufs=6) as xpool, \
             tc.tile_pool(name="st", bufs=3) as spool, \
             tc.tile_pool(name="actt", bufs=3) as apool, \
             tc.tile_pool(name="sums", bufs=3) as supool, \
             tc.tile_pool(name="fin", bufs=1) as fpool, \
             tc.tile_pool(name="ps1", bufs=2, space="PSUM") as ps1pool, \
             tc.tile_pool(name="ps2", bufs=1, space="PSUM") as ps2pool, \
             tc.tile_pool(name="pst", bufs=2, space="PSUM") as pstpool:

            # ---- resident weights: [128, ktile*H] with ktile-major free layout
            w1t_sb = cpool.tile([128, KH * H], DT)
            w2t_sb = cpool.tile([128, KH * H], DT)
            wi1t_sb = cpool.tile([128, KI * H], DT)
            wi2t_sb = cpool.tile([128, KI * H], DT)
            id_sb = cpool.tile([64, 64], DT)
            nc.sync.dma_start(id_sb[:], IDN[:])
            for k in range(KH):
                nc.sync.dma_start(w1t_sb[:, k*H:(k+1)*H], W1T[k*128:(k+1)*128, :])
                nc.sync.dma_start(w2t_sb[:, k*H:(k+1)*H], W2T[k*128:(k+1)*128, :])
            for k in range(KI):
                nc.sync.dma_start(wi1t_sb[:, k*H:(k+1)*H], Wi1T[k*128:(k+1)*128, :])
                nc.sync.dma_start(wi2t_sb[:, k*H:(k+1)*H], Wi2T[k*128:(k+1)*128, :])
            if with_bias:
                bia1_sb = cpool.tile([1, H], DT)
                bia2_sb = cpool.tile([1, H], DT)
                one_sb = cpool.tile([1, BC], DT)
                nc.sync.dma_start(bia1_sb[:], BIA[0:1, :])
                nc.sync.dma_start(bia2_sb[:], BIA[1:2, :])
                nc.sync.dma_start(one_sb[:], ONE[:])

            # ---- XT prefetch
            xts = {}

            def fetch_xt(l):
                if l >= L_steps:
                    return
                t = xpool.tile([128, KI * BC], DT, tag="xt")
                for k in range(KI):
                    nc.sync.dma_start(t[:, k*BC:(k+1)*BC], XT[l, k*128:(k+1)*128, :])
                xts[l] = t

            def groups_for(xt_t, zT, wi_sb, w_sb, bias_row):
                """Per-column-group entry lists: (stationary AP, [bank0, bank1] moving APs)."""
                def xe(k):
                    return (xt_t[:, k*BC:(k+1)*BC],
                            [wi_sb[:, k*H + b*512 : k*H + b*512 + 512] for b in range(2)])
                def ze(k):
                    return (zT[:, k*BC:(k+1)*BC],
                            [w_sb[:, k*H + b*512 : k*H + b*512 + 512] for b in range(2)])
                be = []
                if with_bias:
                    bia_sb = bia1_sb if bias_row == 0 else bia2_sb
                    be = [(one_sb[0:1, :],
                           [bia_sb[0:1, b*512:(b+1)*512] for b in range(2)])]
                if col:
                    return [[xe(0)] + be + [ze(k) for k in range(4)],
                            [xe(1)] + [ze(k) for k in range(4, KH)]]
                return [[xe(0), xe(1)] + be + [ze(k) for k in range(KH)]]

            def emit_mm(ps, groups, i_lo, i_hi):
                """Emit entries [i_lo, i_hi) of each group; start/stop per (group, bank)."""
                for g, entries in enumerate(groups):
                    n = len(entries)
                    tp = (0, g * 64) if col else None
                    rows = ps[g*BC:(g+1)*BC, :] if col else ps[0:BC, :]
                    for i in range(i_lo, min(i_hi, n)):
                        stat, movs = entries[i]
                        for b in range(2):
                            nc.tensor.matmul(
                                rows[:, b*512:(b+1)*512], stat, movs[b],
                                start=(i == 0), stop=(i == n - 1),
                                tile_position=tp)

            # "open" part = the state-independent X entries of the next step's
            # group, emitted early as PE fill for the tanh/add wait (A/B
            # measured: 2 vs 1 saves ~0.4ms total in f32r mode). Must not
            # exceed the per-group X-entry count (col mode has 1 per group),
            # else a stale-state z entry would be emitted before the update.
            N_OPEN = 1 if col else 2
            N_Z2A = 3   # entries per group of the z2 group emitted early (even tail)

            def tanh_step(ps, dst):
                """dst = tanh(pre-activation) for a whole step.

                col mode: the two column-group halves live on different PSUM
                partitions and DVE may read only one PSUM operand, so ACT
                evacuates the high half to SBUF, DVE adds, ACT applies tanh
                (chunked so the three engines pipeline)."""
                if col:
                    bsb = supool.tile([BC, H], F32, tag="bs")
                    s = supool.tile([BC, H], F32, tag="s")
                    for c in range(2):
                        nc.scalar.activation(bsb[:, c*512:(c+1)*512],
                                             ps[BC:2*BC, c*512:(c+1)*512], Copy)
                    for c in range(2):
                        nc.vector.tensor_add(s[:, c*512:(c+1)*512],
                                             ps[0:BC, c*512:(c+1)*512],
                                             bsb[:, c*512:(c+1)*512])
                    for c in range(2):
                        nc.scalar.activation(dst[:, c*512:(c+1)*512],
                                             s[:, c*512:(c+1)*512], Tanh)
                else:
                    for c in range(2):
                        nc.scalar.activation(dst[:, c*512:(c+1)*512],
                                             ps[0:BC, c*512:(c+1)*512], Tanh)

            def z2_post(ps2):
                """tanh + transposes + copy -> new pending z2T tile."""
                z2n = apool.tile([BC, H], DT, tag="z2n")
                tanh_step(ps2, z2n)
                pst2 = pstpool.tile([128, KH * BC], DT, tag="pst")
                for k in range(KH):
                    nc.tensor.transpose(pst2[:, k*BC:(k+1)*BC], z2n[:, k*128:(k+1)*128], id_sb[:])
                z2T_new = spool.tile([128, KH * BC], DT, tag="z2T")
                nc.scalar.activation(z2T_new[:], pst2[:], Copy)
                return z2T_new

            def body():
                nonlocal xts
                xts = {}
                # initial state (zeros, DMA'd so the tiles are typed producers)
                z12T = spool.tile([128, KH * BC], DT, tag="z12T")
                z2T = spool.tile([128, KH * BC], DT, tag="z2T")
                nc.sync.dma_start(z12T[:], ZRO[:])
                nc.sync.dma_start(z2T[:], ZRO[:])
                # prologue: prime XT, open step-0 z1 group, full step-0 z2 group
                for l in range(min(3, L_steps)):
                    fetch_xt(l)
                ps1 = ps1pool.tile([2*BC, H], F32, tag="ps1")
                g1 = groups_for(xts[0], z12T, wi1t_sb, w1t_sb, 0)
                emit_mm(ps1, g1, 0, N_OPEN)
                ps2 = ps2pool.tile([2*BC, H], F32, tag="ps2")
                g2 = groups_for(xts[0], z2T, wi2t_sb, w2t_sb, 1)
                emit_mm(ps2, g2, 0, 99)
                z2T_pending = z2_post(ps2)
                ps2 = g2 = None

                z1n_final = None
                for l in range(L_steps):
                    even = (l % 2 == 0)
                    last = (l == L_steps - 1)
                    fetch_xt(l + 3)

                    # close this step's z1 accumulation (state entering step l)
                    emit_mm(ps1, g1, N_OPEN, 99)

                    # z2 state after step l: updated on even steps
                    if even:
                        z2T = z2T_pending

                    # finish the z2 matmul group for step l+1 (PE fill before tanh wait)
                    if (not last) and (l + 1) % 2 == 0 and ps2 is not None:
                        emit_mm(ps2, g2, N_Z2A, 99)

                    # tanh of this step's z1 (ahead of any z2 ACT work)
                    if last:
                        z1n_f32 = fpool.tile([BC, H], F32)
                        tanh_step(ps1, z1n_f32)
                        z1n_final = z1n_f32
                        break
                    z1n = apool.tile([BC, H], DT, tag="z1n")
                    tanh_step(ps1, z1n)

                    # open next step's z1 group (independent fill before the transposes)
                    ps1 = ps1pool.tile([2*BC, H], F32, tag="ps1")
                    g1 = groups_for(xts[l + 1], z12T, wi1t_sb, w1t_sb, 0)
                    # note: g1 references z12T of step l-1 here only for the X part;
                    # the z entries are re-created below after z12T is updated.
                    emit_mm(ps1, g1, 0, N_OPEN)

                    # transpose z1n
                    pst1 = pstpool.tile([128, KH * BC], DT, tag="pst")
                    for k in range(KH):
                        nc.tensor.transpose(pst1[:, k*BC:(k+1)*BC], z1n[:, k*128:(k+1)*128], id_sb[:])

                    # z2 epilogue for step l+1 (tanh_z2 queues behind tanh_z1 on ACT;
                    # its transposes fill the PE while DVE does the add below)
                    if (not last) and (l + 1) % 2 == 0 and ps2 is not None:
                        z2T_pending = z2_post(ps2)
                        ps2 = g2 = None

                    # z12T = z1nT + z2T(after this step)
                    z12T = spool.tile([128, KH * BC], DT, tag="z12T")
                    for c in range(2):
                        nc.vector.tensor_add(z12T[:, c*256:(c+1)*256], pst1[:, c*256:(c+1)*256], z2T[:, c*256:(c+1)*256])
                    g1 = groups_for(xts[l + 1], z12T, wi1t_sb, w1t_sb, 0)

                    # open the z2 group for step l+2 at the even-step tail
                    # (fills the PE while the add completes)
                    if even and l + 2 < L_steps:
                        ps2 = ps2pool.tile([2*BC, H], F32, tag="ps2")
                        g2 = groups_for(xts[l + 2], z2T, wi2t_sb, w2t_sb, 1)
                        emit_mm(ps2, g2, 0, N_Z2A)

                    if l >= 1:
                        xts.pop(l - 1, None)

                return z1n_final

            if reps > 1:
                with tc.For_i(0, reps, 1):
                    z1n_final = body()
            else:
                z1n_final = body()
            nc.sync.dma_start(OUT[:], z1n_final[:])
    nc.compile()
    return nc


def _get_nc(L_steps, with_bias, reps=1, mode=MODE):
    key = (L_steps, with_bias, reps, mode)
    if key not in _CACHE:
        if isinstance(mode, tuple):
            _CACHE[key] = _build_h(L_steps, with_bias, reps, mode[0], mode[1])
        elif mode in ("v2", "vb2", "v3"):
            _CACHE[key] = _build_v2(L_steps, with_bias, reps, mode)
        elif mode in ("h16", "hb16"):
            _CACHE[key] = _build_h(L_steps, with_bias, reps, mode)
        else:
            _CACHE[key] = _build(L_steps, with_bias, reps, mode)
    return _CACHE[key]


def _np_dt(mode):
    if mode in ("hb16", "vb2"):
        import ml_dtypes
        return ml_dtypes.bfloat16
    return np.float16 if mode in ("f16", "h16", "v2", "v3") else np.float32


def _prep_in_maps(X, W_in1, b_in1, W_rec1, W_in2, b_in2, W_rec2, L_steps, mode=MODE):
    dt = _np_dt(mode)
    with_bias = bool(np.any(b_in1) or np.any(b_in2))
    w1t = np.ascontiguousarray(W_rec1.T.astype(dt))
    w2t = np.ascontiguousarray(W_rec2.T.astype(dt))
    wi1t = np.ascontiguousarray(W_in1.T.astype(dt))
    wi2t = np.ascontiguousarray(W_in2.T.astype(dt))
    if mode in ("h16", "hb16", "v2", "vb2", "v3"):
        idn_key = "IDN2"
        idn = np.ascontiguousarray(np.vstack([np.eye(64, dtype=dt)] * 2))
    else:
        idn_key = "IDN"
        idn = np.eye(64, dtype=dt)
    zro = np.zeros((128, KH * BC), dt)
    in_maps = []
    for c in range(NC):
        xt = np.ascontiguousarray(
            X[c*BC:(c+1)*BC, :L_steps, :].transpose(1, 2, 0).astype(dt))
        m = {"XT": xt, "W1T": w1t, "W2T": w2t, "Wi1T": wi1t, "Wi2T": wi2t,
             idn_key: idn, "ZRO": zro}
        if mode in ("h16", "hb16"):
            m["ID128"] = np.eye(128, dtype=dt)
        if with_bias:
            m["BIA"] = np.ascontiguousarray(
                np.stack([b_in1[:, 0], b_in2[:, 0]]).astype(dt))
            m["ONE"] = np.ones((1, BC), dt)
        in_maps.append(m)
    return in_maps, with_bias


def run_device(X, W_in1, b_in1, W_rec1, W_in2, b_in2, W_rec2, L_steps=L, mode=MODE):
    """Run the recurrence on 8 cores; returns z1_final (B, H) float32."""
    from concourse.bass_utils import run_bass_kernel_spmd
    in_maps, with_bias = _prep_in_maps(X, W_in1, b_in1, W_rec1, W_in2, b_in2,
                                       W_rec2, L_steps, mode)
    nc = _get_nc(L_steps, with_bias, 1, mode)
    res = run_bass_kernel_spmd(nc, in_maps, list(range(NC)))
    return np.concatenate([res.results[c]["OUT"] for c in range(NC)], axis=0)


def kernel(X, W_in1, b_in1, W_rec1, W_in2, b_in2, W_rec2, W_out, b_out):
    X = np.asarray(X); W_out = np.asarray(W_out); b_out = np.asarray(b_out)
    assert X.shape == (B, L, I), f"unexpected X shape {X.shape}"
    z1 = run_device(X, np.asarray(W_in1), np.asarray(b_in1),
                    np.asarray(W_rec1), np.asarray(W_in2), np.asarray(b_in2),
                    np.asarray(W_rec2))
    out = np.tanh(z1.astype(np.float64) @ W_out.astype(np.float64).T
                  + b_out.astype(np.float64)[:, 0])
    return out.reshape(B, 1).astype(np.float32)



# revision 27
# speedup vs baseline: 2.2897x; 1.0072x over previous
"""Trainium2 Bass kernel for nn_AlarmworkRNN: 2-track tanh RNN.

Math (per reference):
  in1 = X @ W_in1.T + b_in1 ; in2 = X @ W_in2.T + b_in2   (folded into recurrence)
  for l in 0..L-1:
      z1n = tanh(in1[l] + (z1 + z2) @ W_rec1.T)
      z2n = tanh(in2[l] + z2 @ W_rec2.T)  if l even else z2
      z1, z2 = z1n, z2n
  out = tanh(z1 @ W_out.T + b_out)       (computed on host, O=1)

Strategy: data-parallel over batch (8 cores x 64 rows). The recurrence state is
held transposed (z12T, z2T: [H=1024 -> 8 k-tiles of 128, B=64]) and used as the
matmul stationary; host-pretransposed weights are the moving operand, resident
in SBUF. The input projection X[l] @ W_in.T is folded into the same PSUM
accumulation as 2 extra k-tiles (stationary = host-pretransposed X[l].T).

Default mode "h16" (fp16, H-SPLIT column tiling): both PE column groups
accumulate ALL 10 k-tiles, but group g streams only the g-th 512-wide half of
the weight columns. The two groups' outputs are disjoint H halves on disjoint
PSUM partition ranges (rows 0:64 = H[0:512], rows 64:128 = H[512:1024]), so
no cross-group reduction is needed: per step just 2 ACT tanh ops, 8 PE
transposes (z1n -> z1nT), and 4 chunked DVE adds (z1nT + z2T -> z12T, chunked
so the next step's first matmuls start as soon as their k-tiles are ready).
The z2 track (updates on even steps only) is computed one step early; its
matmuls/transposes are interleaved as PE fill inside the z1 chain's
tanh/transpose/add latency windows. Measured ~2.6 ms for the full L=512
recurrence on 8 cores (vs 4.8-5.0 ms for the f32r K-split baseline).

HW notes (micro-benchmarked on trn2): col-tiled fp16 FD=512 matmuls stream at
~123 ns/instr (two concurrent 1 elem/cycle group streams; full-width M=128
matmuls are 214 ns -- col tiling IS the 2x); PE transposes ~38 ns batched but
~140 ns when breaking the MM stream; ACT activation [64,512] is ~750 ns
(dtype-independent); DVE adds reading PSUM are ~2.5x slower than SBUF-only
(342 vs 134 ns at FD=128). bfloat16 matmuls are ~12% faster than fp16 but the
full kernel regresses ~18% (slower ACT/DVE path) and error rises to ~1.1e-2.

Other modes kept for reference: "f16" (K-split col tiling, needs an ACT copy +
fp32 DVE add to merge the two half-K partials), "f32r" (no col tiling, ~2x
slower), "hb16" (bf16 H-split).
"""
import numpy as np

B, L, I, H = 512, 512, 256, 1024
NC = 8
BC = B // NC          # 64 batch rows per core
KH = H // 128         # 8 hidden k-tiles
KI = I // 128         # 2 input k-tiles

MODE = "h16"          # "f32r" | "f16" | "h16" | "hb16" | "v2"

# _build_v2 schedule knobs (read at build time; key your cache accordingly)
V2_OPTS = {
    # z2 transpose route:
    #   dma  - xbar transpose DMAs issued at the odd tail (blocks SP queue
    #          head on the z2-tanh dependency)
    #   dma2 - xbar transpose DMAs issued at the next even step (dependency
    #          already met -> no queue-head block); z2split should be 0
    #   dve/gp/act - PE transposes deferred to the even step + PSUM->SBUF
    #          copy on that engine (h16 used dve)
    "z2tr": "dve",
    "z2split": 2,      # z2 state k-tiles emitted at the even tail (0..8)
    "dmaq": "sync",    # engine queue issuing the transpose DMAs: sync|scalar
    "z2stack": 0,      # stacked z2 tanh faults on HW (NRT_EXEC_UNIT) -- keep 0
    "ilv": 1,
}

_CACHE = {}


def _build_h(L_steps, with_bias, reps=1, mode="h16", ablate=None):
    """H-split column-tiled fp16 pipeline.

    Differs from the K-split f16 mode: both PE column groups accumulate ALL
    k-tiles, but group g streams only the g-th 512-wide half of the weight
    columns. The two groups' PSUM outputs are disjoint H halves on disjoint
    partition ranges (rows 0:64 = H[0:512], rows 64:128 = H[512:1024]) -- no
    ACT copy / fp32 DVE add to merge halves, and one PSUM bank per step.
    tanh is 2 ACT ops per step reading the two row ranges.
    """
    import concourse.bacc as bacc
    import concourse.tile as tile
    import concourse.mybir as mybir

    F32 = mybir.dt.float32
    DT = mybir.dt.bfloat16 if mode == "hb16" else mybir.dt.float16
    Tanh = mybir.ActivationFunctionType.Tanh
    Copy = mybir.ActivationFunctionType.Copy

    nc = bacc.Bacc("TRN2", target_bir_lowering=False)
    XT = nc.declare_dram_parameter("XT", [L_steps, I, BC], DT, isOutput=False)
    W1T = nc.declare_dram_parameter("W1T", [H, H], DT, isOutput=False)
    W2T = nc.declare_dram_parameter("W2T", [H, H], DT, isOutput=False)
    Wi1T = nc.declare_dram_parameter("Wi1T", [I, H], DT, isOutput=False)
    Wi2T = nc.declare_dram_parameter("Wi2T", [I, H], DT, isOutput=False)
    IDN = nc.declare_dram_parameter("IDN2", [128, 64], DT, isOutput=False)
    ID128 = nc.declare_dram_parameter("ID128", [128, 128], DT, isOutput=False)
    ZRO = nc.declare_dram_parameter("ZRO", [128, KH * BC], DT, isOutput=False)
    if with_bias:
        BIA = nc.declare_dram_parameter("BIA", [2, H], DT, isOutput=False)
        ONE = nc.declare_dram_parameter("ONE", [1, BC], DT, isOutput=False)
    OUT = nc.declare_dram_parameter("OUT", [BC, H], F32, isOutput=True)

    with tile.TileContext(nc) as tc:
        with tc.tile_pool(name="const", bufs=1) as cpool, \
             tc.tile_pool(name="xt", bufs=6) as xpool, \
             tc.tile_pool(name="st", bufs=4) as spool, \
             tc.tile_pool(name="actt", bufs=3) as apool, \
             tc.tile_pool(name="fin", bufs=1) as fpool, \
             tc.tile_pool(name="ps1", bufs=2, space="PSUM") as ps1pool, \
             tc.tile_pool(name="ps2", bufs=2, space="PSUM") as ps2pool, \
             tc.tile_pool(name="pst", bufs=(1 if ablate == "notr" else 3), space="PSUM") as pstpool:

            # ---- resident weights: [128, ktile*H] with ktile-major free layout
            w1t_sb = cpool.tile([128, KH * H], DT)
            w2t_sb = cpool.tile([128, KH * H], DT)
            wi1t_sb = cpool.tile([128, KI * H], DT)
            wi2t_sb = cpool.tile([128, KI * H], DT)
            id_sb = cpool.tile([128, 64], DT)
            id128_sb = cpool.tile([128, 128], DT)
            nc.sync.dma_start(id_sb[:], IDN[:])
            nc.sync.dma_start(id128_sb[:], ID128[:])
            for k in range(KH):
                nc.sync.dma_start(w1t_sb[:, k*H:(k+1)*H], W1T[k*128:(k+1)*128, :])
                nc.sync.dma_start(w2t_sb[:, k*H:(k+1)*H], W2T[k*128:(k+1)*128, :])
            for k in range(KI):
                nc.sync.dma_start(wi1t_sb[:, k*H:(k+1)*H], Wi1T[k*128:(k+1)*128, :])
                nc.sync.dma_start(wi2t_sb[:, k*H:(k+1)*H], Wi2T[k*128:(k+1)*128, :])
            if with_bias:
                bia1_sb = cpool.tile([1, H], DT)
                bia2_sb = cpool.tile([1, H], DT)
                one_sb = cpool.tile([1, BC], DT)
                nc.sync.dma_start(bia1_sb[:], BIA[0:1, :])
                nc.sync.dma_start(bia2_sb[:], BIA[1:2, :])
                nc.sync.dma_start(one_sb[:], ONE[:])

            xts = {}

            def fetch_xt(l):
                if l >= L_steps:
                    return
                t = xpool.tile([128, KI * BC], DT, tag="xt")
                for k in range(KI):
                    nc.sync.dma_start(t[:, k*BC:(k+1)*BC], XT[l, k*128:(k+1)*128, :])
                xts[l] = t

            def emit_x(ps, xt_t, wi_sb, bias_row):
                """Open both groups' accumulations with the X (+bias) entries."""
                for i in range(KI):
                    for g in range(2):
                        nc.tensor.matmul(
                            ps[g*BC:(g+1)*BC, :],
                            xt_t[:, i*BC:(i+1)*BC],
                            wi_sb[:, i*H + g*512 : i*H + g*512 + 512],
                            start=(i == 0), stop=False,
                            tile_position=(0, g*64))
                if with_bias:
                    bia_sb = bia1_sb if bias_row == 0 else bia2_sb
                    for g in range(2):
                        nc.tensor.matmul(
                            ps[g*BC:(g+1)*BC, :],
                            one_sb[0:1, :],
                            bia_sb[0:1, g*512:(g+1)*512],
                            start=False, stop=False,
                            tile_position=(0, g*64))

            def emit_z(ps, zT, w_sb, k_lo, k_hi):
                """State entries k_lo..k_hi-1 for both groups; stop on k==KH-1."""
                for k in range(k_lo, k_hi):
                    for g in range(2):
                        nc.tensor.matmul(
                            ps[g*BC:(g+1)*BC, :],
                            zT[:, k*BC:(k+1)*BC],
                            w_sb[:, k*H + g*512 : k*H + g*512 + 512],
                            start=False, stop=(k == KH - 1),
                            tile_position=(0, g*64))

            def tanh_g(ps, dst, g, dt_hint=None):
                nc.scalar.activation(dst[:, g*512:(g+1)*512],
                                     ps[g*BC:(g+1)*BC, :], Tanh)

            def transposes(src, pst, k_lo, k_hi):
                """PE stationary reads must be partition-base-0: src is the
                [64, 1024] activation tile, k-tile k at cols 128*k."""
                for k in range(k_lo, k_hi):
                    nc.tensor.transpose(pst[:, k*BC:(k+1)*BC],
                                        src[:, k*128:(k+1)*128], id_sb[0:64, :])

            def inject(pst, zT):
                """pst = zT via identity matmul (out = I128.T @ zT), opening
                an accumulation the transposes then add z1nT onto: replaces
                the 4 PSUM-sourced DVE adds with 2 plain copies."""
                nc.tensor.matmul(pst[:, :], id128_sb[:, :], zT[:, :],
                                 start=True, stop=False, skip_group_check=True)

            def transposes_acc(src, pst, k_lo, k_hi):
                for k in range(k_lo, k_hi):
                    nc.tensor.matmul(pst[:, k*BC:(k+1)*BC],
                                     src[:, k*128:(k+1)*128], id_sb[0:64, :],
                                     is_transpose=True, start=False,
                                     stop=(k == KH - 1), skip_group_check=True)

            def z2_dma_transposes(z2n2, dst):
                """z2nT via the DMA xbar from the stacked [128, 512] tanh
                output (k-tile k at rows 64*(k//4), cols 128*(k%4)): DMA can
                read any partition base, and the z2 track has ~1.5 steps of
                slack, so the latency is free and PE/ACT/DVE all shed work."""
                for k in range(KH):
                    r, c = 64 * (k // 4), 128 * (k % 4)
                    nc.sync.dma_start_transpose(dst[:, k*BC:(k+1)*BC],
                                                z2n2[r:r+64, c:c+128])

            def z2_epilogue(z2n):
                """Transposes + copy for a z2n computed at the prior odd step.

                Emitted at the FOLLOWING even step, right after that step's
                critical z1 matmuls: the 8 transposes then execute during the
                z1-tanh ACT window (their z2n dependency is long satisfied),
                acting as PE fill instead of stalling the next step."""
                pst2 = pstpool.tile([128, KH * BC], DT, tag="pst")
                transposes(z2n, pst2, 0, KH)
                z2T_new = spool.tile([128, KH * BC], DT, tag="z2Tp")
                nc.vector.tensor_copy(z2T_new[:], pst2[:])
                return z2T_new

            def body():
                nonlocal xts
                xts = {}
                z12T = spool.tile([128, KH * BC], DT, tag="z12T")
                z2T = spool.tile([128, KH * BC], DT, tag="z2T")
                nc.sync.dma_start(z12T[:], ZRO[:])
                nc.sync.dma_start(z2T[:], ZRO[:])
                for l in range(min(3, L_steps)):
                    fetch_xt(l)
                # prologue: step-0 z1 opened with X; step-0 z2 fully emitted
                ps1 = ps1pool.tile([2*BC, 512], F32, tag="ps1")
                emit_x(ps1, xts[0], wi1t_sb, 0)
                ps2 = ps2pool.tile([2*BC, 512], F32, tag="ps2")
                emit_x(ps2, xts[0], wi2t_sb, 1)
                emit_z(ps2, z2T, w2t_sb, 0, KH)
                z2n = apool.tile([BC, H], DT, tag="z2n")
                tanh_g(ps2, z2n, 0)
                tanh_g(ps2, z2n, 1)
                z2T_pending = z2_epilogue(z2n)
                ps2 = None
                z2_open = False
                z2n_deferred = None
                z1n_const = None         # timing-ablation stand-ins
                if ablate in ("notanh", "notr"):
                    z1n_const = apool.tile([BC, H], DT, tag="z1n")
                    tanh_g(ps2, z1n_const, 0)
                    tanh_g(ps2, z1n_const, 1)
                if ablate == "notr":
                    pst_const = pstpool.tile([128, KH * BC], DT, tag="pstc")
                    transposes(z1n_const, pst_const, 0, KH)

                z1n_final = None
                z1n_final = None
                for l in range(L_steps):
                    even = (l % 2 == 0)
                    last = (l == L_steps - 1)
                    fetch_xt(l + 3)

                    # this step's z1 matmuls (state entering step l)
                    emit_z(ps1, z12T, w1t_sb, 0, KH)

                    if last:
                        z1n_f32 = fpool.tile([BC, H], F32)
                        tanh_g(ps1, z1n_f32, 0)
                        tanh_g(ps1, z1n_f32, 1)
                        z1n_final = z1n_f32
                        break

                    if ablate == "nopost":
                        # PE stream only: next accumulation, constant state
                        ps1 = ps1pool.tile([2*BC, 512], F32, tag="ps1")
                        emit_x(ps1, xts[l + 1], wi1t_sb, 0)
                        if (not even) and z2_open:
                            emit_z(ps2, z2T, w2t_sb, 0, KH)
                            ps2 = None
                            z2_open = False
                        if even and l + 2 < L_steps:
                            ps2 = ps2pool.tile([2*BC, 512], F32, tag="ps2")
                            emit_x(ps2, xts[l + 2], wi2t_sb, 1)
                            z2_open = True
                        if l >= 1:
                            xts.pop(l - 1, None)
                        continue

                    # tanh of this step's z1. g0 is the critical producer
                    # (transposes k0-3 wait on it), so it is split into two
                    # FD=256 ACT ops (~400ns each vs ~750ns for FD=512): the
                    # first transposes and state-add chunks start earlier.
                    if ablate != "notanh":
                        z1n = apool.tile([BC, H], DT, tag="z1n")
                        nc.scalar.activation(z1n[:, 0:256],
                                             ps1[0:BC, 0:256], Tanh)
                        nc.scalar.activation(z1n[:, 256:512],
                                             ps1[0:BC, 256:512], Tanh)
                        tanh_g(ps1, z1n, 1)
                        z1n_cur = z1n
                    else:
                        z1n_cur = z1n_const

                    # --- PE fill during the tanh-g0 ACT window ---
                    if even:
                        if z2n_deferred is not None:
                            # deferred z2 epilogue from the prior odd step
                            z2T_pending = z2_epilogue(z2n_deferred)
                            z2n_deferred = None
                        # z2 state after step l
                        if ablate != "norec":
                            z2T = z2T_pending
                    elif z2_open:
                        # first chunk of the z2 group for step l+1
                        emit_z(ps2, z2T, w2t_sb, 0, 3)

                    if even and l + 2 < L_steps:
                        # open the z2 group for step l+2 here: extra MM fill
                        # for the tanh window (its stationary is X only)
                        ps2 = ps2pool.tile([2*BC, 512], F32, tag="ps2")
                        emit_x(ps2, xts[l + 2], wi2t_sb, 1)
                        z2_open = True

                    # transposes chase the tanh chunks: k0-1 after the first
                    # FD=256 op, k2-3 after the second, k4-7 after g1
                    pst1 = pstpool.tile([128, KH * BC], DT, tag="pst")
                    transposes(z1n_cur, pst1, 0, 2)
                    transposes(z1n_cur, pst1, 2, 4)

                    ps1 = ps1pool.tile([2*BC, 512], F32, tag="ps1")
                    emit_x(ps1, xts[l + 1], wi1t_sb, 0)

                    # transposes k4..7 (wait on tanh g1)
                    transposes(z1n_cur, pst1, 4, KH)

                    # z12T = z1nT + z2T (chunked so next-step MMs start early)
                    z12T_new = spool.tile([128, KH * BC], DT, tag="z12T")
                    if ablate == "norec":
                        # break the DVE->MM edge: state stays the zero tile
                        trash = z12T_new
                        z12T_new = z12T
                    add_dst = trash if ablate == "norec" else z12T_new
                    for c in range(4):
                        nc.vector.tensor_add(add_dst[:, c*128:(c+1)*128],
                                             pst1[:, c*128:(c+1)*128],
                                             z2T[:, c*128:(c+1)*128])

                    # --- remaining fills, executing during the add latency ---
                    if (not even) and z2_open:
                        emit_z(ps2, z2T, w2t_sb, 3, KH)
                        if ablate not in ("notanh", "notr"):
                            z2n = apool.tile([BC, H], DT, tag="z2n")
                            tanh_g(ps2, z2n, 0)
                            tanh_g(ps2, z2n, 1)
                            z2n_deferred = z2n  # transposes+copy at step l+1
                        ps2 = None
                        z2_open = False

                    z12T = z12T_new
                    if l >= 1:
                        xts.pop(l - 1, None)

                return z1n_final

            if reps > 1:
                with tc.For_i(0, reps, 1):
                    z1n_final = body()
            else:
                z1n_final = body()
            nc.sync.dma_start(OUT[:], z1n_final[:])
    nc.compile()
    return nc


def _build_v2(L_steps, with_bias, reps=1, mode="v2"):
    """Balanced-parity fp16 H-split pipeline (evolution of h16).

    Two changes vs _build_h:
    1. The z2 track's transposes ride the DMA crossbar instead of the PE:
       one dma_start_transpose per tanh half, using a 3D blocked-transpose
       destination AP ([128, k, 64] <- [64, k*128]), shedding 8 PE
       transposes + a [128,512] PSUM-sourced DVE copy per step pair. DVE
       then only does the 4 z12T add chunks, so the even-step adds are
       never queued behind the copy (which previously stalled the next
       step's first matmul by ~0.8us on even steps).
    2. The z2 matmul group is split 10/10 across the step pair (x-open +
       state k0-2 at the even tail once the DMA'd z2T lands, k3-7 at the
       odd tail), so both parities carry ~30 matmul instrs of PE fill
       around the step boundary instead of 20/40.
    """
    import concourse.bacc as bacc
    import concourse.tile as tile
    import concourse.mybir as mybir

    F32 = mybir.dt.float32
    DT = mybir.dt.bfloat16 if mode == "vb2" else mybir.dt.float16
    # v3m: moving weight operands in bf16 (mixed-dtype matmul, ~12% faster
    # stream per prior micro-bench; stationary/state stay fp16 so the
    # ACT/DVE path is untouched and only weight quantization changes)
    DTW = mybir.dt.bfloat16 if mode == "v3m" else DT
    Tanh = mybir.ActivationFunctionType.Tanh
    # v3: stacked activations -- tanh reads the full [128, .] PSUM range
    # (both column groups at once; ACT cost is free-dim-bound, partitions
    # are free) and the k4-7 transposes read their stationary at partition
    # base 64 with tile_position=(64, 0)
    stack = mode in ("v3", "v3m")

    nc = bacc.Bacc("TRN2", target_bir_lowering=False)
    XT = nc.declare_dram_parameter("XT", [L_steps, I, BC], DT, isOutput=False)
    W1T = nc.declare_dram_parameter("W1T", [H, H], DTW, isOutput=False)
    W2T = nc.declare_dram_parameter("W2T", [H, H], DTW, isOutput=False)
    Wi1T = nc.declare_dram_parameter("Wi1T", [I, H], DTW, isOutput=False)
    Wi2T = nc.declare_dram_parameter("Wi2T", [I, H], DTW, isOutput=False)
    IDN = nc.declare_dram_parameter("IDN2", [128, 64], DT, isOutput=False)
    ZRO = nc.declare_dram_parameter("ZRO", [128, KH * BC], DT, isOutput=False)
    if with_bias:
        BIA = nc.declare_dram_parameter("BIA", [2, H], DTW, isOutput=False)
        ONE = nc.declare_dram_parameter("ONE", [1, BC], DT, isOutput=False)
    OUT = nc.declare_dram_parameter("OUT", [BC, H], F32, isOutput=True)

    with tile.TileContext(nc) as tc:
        with tc.tile_pool(name="const", bufs=1) as cpool, \
             tc.tile_pool(name="xt", bufs=6) as xpool, \
             tc.tile_pool(name="st", bufs=6) as spool, \
             tc.tile_pool(name="actt", bufs=4) as apool, \
             tc.tile_pool(name="fin", bufs=1) as fpool, \
             tc.tile_pool(name="ps1", bufs=2, space="PSUM") as ps1pool, \
             tc.tile_pool(name="ps2", bufs=2, space="PSUM") as ps2pool, \
             tc.tile_pool(name="pst", bufs=2, space="PSUM") as pstpool:

            w1t_sb = cpool.tile([128, KH * H], DTW)
            w2t_sb = cpool.tile([128, KH * H], DTW)
            wi1t_sb = cpool.tile([128, KI * H], DTW)
            wi2t_sb = cpool.tile([128, KI * H], DTW)
            id_sb = cpool.tile([128, 64], DT)
            nc.sync.dma_start(id_sb[:], IDN[:])
            for k in range(KH):
                nc.sync.dma_start(w1t_sb[:, k*H:(k+1)*H], W1T[k*128:(k+1)*128, :])
                nc.sync.dma_start(w2t_sb[:, k*H:(k+1)*H], W2T[k*128:(k+1)*128, :])
            for k in range(KI):
                nc.sync.dma_start(wi1t_sb[:, k*H:(k+1)*H], Wi1T[k*128:(k+1)*128, :])
                nc.sync.dma_start(wi2t_sb[:, k*H:(k+1)*H], Wi2T[k*128:(k+1)*128, :])
            if with_bias:
                bia1_sb = cpool.tile([1, H], DTW)
                bia2_sb = cpool.tile([1, H], DTW)
                one_sb = cpool.tile([1, BC], DT)
                nc.sync.dma_start(bia1_sb[:], BIA[0:1, :])
                nc.sync.dma_start(bia2_sb[:], BIA[1:2, :])
                nc.sync.dma_start(one_sb[:], ONE[:])

            xts = {}

            def fetch_xt(l):
                if l >= L_steps:
                    return
                t = xpool.tile([128, KI * BC], DT, tag="xt")
                for k in range(KI):
                    nc.sync.dma_start(t[:, k*BC:(k+1)*BC], XT[l, k*128:(k+1)*128, :])
                xts[l] = t

            def emit_x(ps, xt_t, wi_sb, bias_row):
                for i in range(KI):
                    for g in range(2):
                        nc.tensor.matmul(
                            ps[g*BC:(g+1)*BC, :],
                            xt_t[:, i*BC:(i+1)*BC],
                            wi_sb[:, i*H + g*512 : i*H + g*512 + 512],
                            start=(i == 0), stop=False,
                            tile_position=(0, g*64))
                if with_bias:
                    bia_sb = bia1_sb if bias_row == 0 else bia2_sb
                    for g in range(2):
                        nc.tensor.matmul(
                            ps[g*BC:(g+1)*BC, :],
                            one_sb[0:1, :],
                            bia_sb[0:1, g*512:(g+1)*512],
                            start=False, stop=False,
                            tile_position=(0, g*64))

            def emit_z(ps, zT, w_sb, k_lo, k_hi):
                for k in range(k_lo, k_hi):
                    for g in range(2):
                        nc.tensor.matmul(
                            ps[g*BC:(g+1)*BC, :],
                            zT[:, k*BC:(k+1)*BC],
                            w_sb[:, k*H + g*512 : k*H + g*512 + 512],
                            start=False, stop=(k == KH - 1),
                            tile_position=(0, g*64))

            def tanh_g(ps, dst, g):
                nc.scalar.activation(dst[:, g*512:(g+1)*512],
                                     ps[g*BC:(g+1)*BC, :], Tanh)

            def transposes(src, pst, k_lo, k_hi):
                for k in range(k_lo, k_hi):
                    nc.tensor.transpose(pst[:, k*BC:(k+1)*BC],
                                        src[:, k*128:(k+1)*128], id_sb[0:64, :])

            def transposes_s(src, pst, ks):
                """Transposes from the stacked [128, 512] activation tile:
                k0-3 live on partitions 0:64, k4-7 on 64:128 (stationary at
                partition base 64, a legal 64-row tile position)."""
                for k in ks:
                    if k < 4:
                        nc.tensor.transpose(pst[:, k*BC:(k+1)*BC],
                                            src[0:64, k*128:(k+1)*128],
                                            id_sb[0:64, :])
                    else:
                        c = (k - 4) * 128
                        nc.tensor.transpose(pst[:, k*BC:(k+1)*BC],
                                            src[64:128, c:c+128],
                                            id_sb[64:128, :])

            z2tr = V2_OPTS.get("z2tr", "dma")
            z2split = V2_OPTS.get("z2split", 3)
            dmaq = nc.scalar if V2_OPTS.get("dmaq") == "scalar" else nc.sync

            def dma_tr(dst, src, k_lo, k_hi):
                """dst[:, k*BC:(k+1)*BC] = src[:, k*128:(k+1)*128].T for
                k in [k_lo, k_hi) -- one blocked-transpose DMA: the 3D
                destination AP distributes the 128-row blocks of the
                conceptual [512, 64] transpose k-major (same idiom as
                tile_matmul's kxn_tile xbar path)."""
                view = dst[:, k_lo*BC:k_hi*BC].rearrange(
                    "p (k b) -> p k b", b=BC)
                dmaq.dma_start_transpose(
                    view, src[0:BC, k_lo*128:k_hi*128])

            def pe_tr_epilogue(z2n_src):
                """h16-style: PE transposes + PSUM->SBUF copy -> new z2T."""
                pst2 = pstpool.tile([128, KH * BC], DT, tag="pst")
                if stack and V2_OPTS.get("z2stack", 1):
                    transposes_s(z2n_src, pst2, range(KH))
                else:
                    transposes(z2n_src, pst2, 0, KH)
                z2T_new = spool.tile([128, KH * BC], DT, tag="z2T")
                if z2tr == "gp":
                    nc.gpsimd.tensor_copy(z2T_new[:], pst2[:])
                elif z2tr == "act":
                    nc.scalar.activation(
                        z2T_new[:], pst2[:],
                        mybir.ActivationFunctionType.Copy)
                else:
                    nc.vector.tensor_copy(z2T_new[:], pst2[:])
                return z2T_new

            def body():
                nonlocal xts
                xts = {}
                z12T = spool.tile([128, KH * BC], DT, tag="z12T")
                z2T = spool.tile([128, KH * BC], DT, tag="z2T")
                nc.sync.dma_start(z12T[:], ZRO[:])
                nc.sync.dma_start(z2T[:], ZRO[:])
                for l in range(min(3, L_steps)):
                    fetch_xt(l)
                # prologue: step-0 z1 opened with X; step-0 z2 fully emitted,
                # tanh'd, and sent through the DMA transpose
                ps1 = ps1pool.tile([2*BC, 512], F32, tag="ps1")
                emit_x(ps1, xts[0], wi1t_sb, 0)
                ps2 = ps2pool.tile([2*BC, 512], F32, tag="ps2")
                emit_x(ps2, xts[0], wi2t_sb, 1)
                emit_z(ps2, z2T, w2t_sb, 0, KH)
                z2n_deferred = None
                z2T_pending = None
                z2stack = stack and V2_OPTS.get("z2stack", 1)
                if z2stack:
                    z2n = apool.tile([128, 512], DT, tag="z2n")
                    nc.scalar.activation(z2n[:, 0:256], ps2[0:128, 0:256], Tanh)
                    nc.scalar.activation(z2n[:, 256:512], ps2[0:128, 256:512], Tanh)
                    z2n_deferred = z2n
                elif z2tr == "dma":
                    z2n = apool.tile([BC, H], DT, tag="z2n")
                    z2T_pending = spool.tile([128, KH * BC], DT, tag="z2T")
                    tanh_g(ps2, z2n, 0)
                    dma_tr(z2T_pending, z2n, 0, 4)
                    tanh_g(ps2, z2n, 1)
                    dma_tr(z2T_pending, z2n, 4, KH)
                else:
                    z2n = apool.tile([BC, H], DT, tag="z2n")
                    tanh_g(ps2, z2n, 0)
                    tanh_g(ps2, z2n, 1)
                    z2n_deferred = z2n
                ps2 = None
                z2_open = False

                z1n_final = None
                for l in range(L_steps):
                    even = (l % 2 == 0)
                    last = (l == L_steps - 1)
                    fetch_xt(l + 3)

                    # this step's z1 state matmuls
                    emit_z(ps1, z12T, w1t_sb, 0, KH)

                    # z2 group work at the step tails (PE fill):
                    if even:
                        if l + 2 < L_steps:
                            ps2 = ps2pool.tile([2*BC, 512], F32, tag="ps2")
                            emit_x(ps2, xts[l + 2], wi2t_sb, 1)
                            z2_open = True
                        # materialize z2 state after step l as z2T
                        if z2tr == "dma":
                            z2T = z2T_pending
                        elif z2tr == "dma2":
                            z2T_new = spool.tile([128, KH * BC], DT, tag="z2T")
                            dma_tr(z2T_new, z2n_deferred, 0, 4)
                            dma_tr(z2T_new, z2n_deferred, 4, KH)
                            z2n_deferred = None
                            z2T = z2T_new
                        else:
                            z2T = pe_tr_epilogue(z2n_deferred)
                            z2n_deferred = None
                        if z2_open and z2split > 0:
                            emit_z(ps2, z2T, w2t_sb, 0, z2split)
                    elif z2_open:
                        emit_z(ps2, z2T, w2t_sb, z2split, KH)

                    if last:
                        z1n_f32 = fpool.tile([BC, H], F32)
                        tanh_g(ps1, z1n_f32, 0)
                        tanh_g(ps1, z1n_f32, 1)
                        z1n_final = z1n_f32
                        break

                    if stack:
                        # z1 tanh over the full 128-partition PSUM range in
                        # 2x FD=256 ops: op1 delivers k0-1 AND k4-5, op2 the
                        # rest -- half the ACT latency of the g-shifted form
                        z1n = apool.tile([128, 512], DT, tag="z1n")
                        nc.scalar.activation(z1n[:, 0:256],
                                             ps1[0:128, 0:256], Tanh)
                        nc.scalar.activation(z1n[:, 256:512],
                                             ps1[0:128, 256:512], Tanh)
                    else:
                        z1n = apool.tile([BC, H], DT, tag="z1n")
                        nc.scalar.activation(z1n[:, 0:256],
                                             ps1[0:BC, 0:256], Tanh)
                        nc.scalar.activation(z1n[:, 256:512],
                                             ps1[0:BC, 256:512], Tanh)
                        tanh_g(ps1, z1n, 1)

                    # z2 tanh (+ DMA transposes) -- odd steps; group closed
                    if (not even) and z2_open:
                        if z2stack:
                            z2n = apool.tile([128, 512], DT, tag="z2n")
                            nc.scalar.activation(z2n[:, 0:256], ps2[0:128, 0:256], Tanh)
                            nc.scalar.activation(z2n[:, 256:512], ps2[0:128, 256:512], Tanh)
                            z2n_deferred = z2n
                        elif z2tr == "dma":
                            z2n = apool.tile([BC, H], DT, tag="z2n")
                            z2T_new = spool.tile([128, KH * BC], DT, tag="z2T")
                            tanh_g(ps2, z2n, 0)
                            dma_tr(z2T_new, z2n, 0, 4)
                            tanh_g(ps2, z2n, 1)
                            dma_tr(z2T_new, z2n, 4, KH)
                            z2T_pending = z2T_new
                        else:
                            z2n = apool.tile([BC, H], DT, tag="z2n")
                            tanh_g(ps2, z2n, 0)
                            tanh_g(ps2, z2n, 1)
                            z2n_deferred = z2n
                        ps2 = None
                        z2_open = False

                    pst1 = pstpool.tile([128, KH * BC], DT, tag="pst")
                    z12T_new = spool.tile([128, KH * BC], DT, tag="z12T")

                    def add_c(c):
                        nc.vector.tensor_add(z12T_new[:, c*128:(c+1)*128],
                                             pst1[:, c*128:(c+1)*128],
                                             z2T[:, c*128:(c+1)*128])

                    if stack and V2_OPTS.get("ilv", 1):
                        # per-chunk: transpose pair -> its add, ordered by
                        # tanh-op readiness (op1: k0-1,k4-5; op2: k2-3,k6-7)
                        transposes_s(z1n, pst1, [0, 1])
                        add_c(0)
                        transposes_s(z1n, pst1, [4, 5])
                        add_c(2)
                        ps1 = ps1pool.tile([2*BC, 512], F32, tag="ps1")
                        emit_x(ps1, xts[l + 1], wi1t_sb, 0)
                        transposes_s(z1n, pst1, [2, 3])
                        add_c(1)
                        transposes_s(z1n, pst1, [6, 7])
                        add_c(3)
                    elif stack:
                        transposes_s(z1n, pst1, [0, 1, 2, 3])
                        ps1 = ps1pool.tile([2*BC, 512], F32, tag="ps1")
                        emit_x(ps1, xts[l + 1], wi1t_sb, 0)
                        transposes_s(z1n, pst1, [4, 5, 6, 7])
                        for c in range(4):
                            add_c(c)
                    else:
                        transposes(z1n, pst1, 0, 2)
                        transposes(z1n, pst1, 2, 4)
                        ps1 = ps1pool.tile([2*BC, 512], F32, tag="ps1")
                        emit_x(ps1, xts[l + 1], wi1t_sb, 0)
                        transposes(z1n, pst1, 4, KH)
                        for c in range(4):
                            add_c(c)
                    z12T = z12T_new
                    if l >= 1:
                        xts.pop(l - 1, None)

                return z1n_final

            if reps > 1:
                with tc.For_i(0, reps, 1):
                    z1n_final = body()
            else:
                z1n_final = body()
            nc.sync.dma_start(OUT[:], z1n_final[:])
    nc.compile()
    return nc


def _build(L_steps, with_bias, reps=1, mode=MODE):
    import concourse.bacc as bacc
    import concourse.tile as tile
    import concourse.mybir as mybir

    F32 = mybir.dt.float32
    DT = mybir.dt.float16 if mode == "f16" else mybir.dt.float32r
    col = (mode == "f16")
    Tanh = mybir.ActivationFunctionType.Tanh
    Copy = mybir.ActivationFunctionType.Copy

    nc = bacc.Bacc("TRN2", target_bir_lowering=False)
    XT = nc.declare_dram_parameter("XT", [L_steps, I, BC], DT, isOutput=False)
    W1T = nc.declare_dram_parameter("W1T", [H, H], DT, isOutput=False)
    W2T = nc.declare_dram_parameter("W2T", [H, H], DT, isOutput=False)
    Wi1T = nc.declare_dram_parameter("Wi1T", [I, H], DT, isOutput=False)
    Wi2T = nc.declare_dram_parameter("Wi2T", [I, H], DT, isOutput=False)
    IDN = nc.declare_dram_parameter("IDN", [64, 64], DT, isOutput=False)
    ZRO = nc.declare_dram_parameter("ZRO", [128, KH * BC], DT, isOutput=False)
    if with_bias:
        BIA = nc.declare_dram_parameter("BIA", [2, H], DT, isOutput=False)
        ONE = nc.declare_dram_parameter("ONE", [1, BC], DT, isOutput=False)
    OUT = nc.declare_dram_parameter("OUT", [BC, H], F32, isOutput=True)

    with tile.TileContext(nc) as tc:
        with tc.tile_pool(name="const", bufs=1) as cpool, \
             tc.tile_pool(name="xt", bufs=6) as xpool, \
             tc.tile_pool(name="st", bufs=3) as spool, \
             tc.tile_pool(name="actt", bufs=3) as apool, \
             tc.tile_pool(name="sums", bufs=3) as supool, \
             tc.tile_pool(name="fin", bufs=1) as fpool, \
             tc.tile_pool(name="ps1", bufs=2, space="PSUM") as ps1pool, \
             tc.tile_pool(name="ps2", bufs=1, space="PSUM") as ps2pool, \
             tc.tile_pool(name="pst", bufs=2, space="PSUM") as pstpool:

            # ---- resident weights: [128, ktile*H] with ktile-major free layout
            w1t_sb = cpool.tile([128, KH * H], DT)
            w2t_sb = cpool.tile([128, KH * H], DT)
            wi1t_sb = cpool.tile([128, KI * H], DT)
            wi2t_sb = cpool.tile([128, KI * H], DT)
            id_sb = cpool.tile([64, 64], DT)
            nc.sync.dma_start(id_sb[:], IDN[:])
            for k in range(KH):
                nc.sync.dma_start(w1t_sb[:, k*H:(k+1)*H], W1T[k*128:(k+1)*128, :])
                nc.sync.dma_start(w2t_sb[:, k*H:(k+1)*H], W2T[k*128:(k+1)*128, :])
            for k in range(KI):
                nc.sync.dma_start(wi1t_sb[:, k*H:(k+1)*H], Wi1T[k*128:(k+1)*128, :])
                nc.sync.dma_start(wi2t_sb[:, k*H:(k+1)*H], Wi2T[k*128:(k+1)*128, :])
            if with_bias:
                bia1_sb = cpool.tile([1, H], DT)
                bia2_sb = cpool.tile([1, H], DT)
                one_sb = cpool.tile([1, BC], DT)
                nc.sync.dma_start(bia1_sb[:], BIA[0:1, :])
                nc.sync.dma_start(bia2_sb[:], BIA[1:2, :])
                nc.sync.dma_start(one_sb[:], ONE[:])

            # ---- XT prefetch
            xts = {}

            def fetch_xt(l):
                if l >= L_steps:
                    return
                t = xpool.tile([128, KI * BC], DT, tag="xt")
                for k in range(KI):
                    nc.sync.dma_start(t[:, k*BC:(k+1)*BC], XT[l, k*128:(k+1)*128, :])
                xts[l] = t

            def groups_for(xt_t, zT, wi_sb, w_sb, bias_row):
                """Per-column-group entry lists: (stationary AP, [bank0, bank1] moving APs)."""
                def xe(k):
                    return (xt_t[:, k*BC:(k+1)*BC],
                            [wi_sb[:, k*H + b*512 : k*H + b*512 + 512] for b in range(2)])
                def ze(k):
                    return (zT[:, k*BC:(k+1)*BC],
                            [w_sb[:, k*H + b*512 : k*H + b*512 + 512] for b in range(2)])
                be = []
                if with_bias:
                    bia_sb = bia1_sb if bias_row == 0 else bia2_sb
                    be = [(one_sb[0:1, :],
                           [bia_sb[0:1, b*512:(b+1)*512] for b in range(2)])]
                if col:
                    return [[xe(0)] + be + [ze(k) for k in range(4)],
                            [xe(1)] + [ze(k) for k in range(4, KH)]]
                return [[xe(0), xe(1)] + be + [ze(k) for k in range(KH)]]

            def emit_mm(ps, groups, i_lo, i_hi):
                """Emit entries [i_lo, i_hi) of each group; start/stop per (group, bank)."""
                for g, entries in enumerate(groups):
                    n = len(entries)
                    tp = (0, g * 64) if col else None
                    rows = ps[g*BC:(g+1)*BC, :] if col else ps[0:BC, :]
                    for i in range(i_lo, min(i_hi, n)):
                        stat, movs = entries[i]
                        for b in range(2):
                            nc.tensor.matmul(
                                rows[:, b*512:(b+1)*512], stat, movs[b],
                                start=(i == 0), stop=(i == n - 1),
                                tile_position=tp)

            # "open" part = the state-independent X entries of the next step's
            # group, emitted early as PE fill for the tanh/add wait (A/B
            # measured: 2 vs 1 saves ~0.4ms total in f32r mode). Must not
            # exceed the per-group X-entry count (col mode has 1 per group),
            # else a stale-state z entry would be emitted before the update.
            N_OPEN = 1 if col else 2
            N_Z2A = 3   # entries per group of the z2 group emitted early (even tail)

            def tanh_step(ps, dst):
                """dst = tanh(pre-activation) for a whole step.

                col mode: the two column-group halves live on different PSUM
                partitions and DVE may read only one PSUM operand, so ACT
                evacuates the high half to SBUF, DVE adds, ACT applies tanh
                (chunked so the three engines pipeline)."""
                if col:
                    bsb = supool.tile([BC, H], F32, tag="bs")
                    s = supool.tile([BC, H], F32, tag="s")
                    for c in range(2):
                        nc.scalar.activation(bsb[:, c*512:(c+1)*512],
                                             ps[BC:2*BC, c*512:(c+1)*512], Copy)
                    for c in range(2):
                        nc.vector.tensor_add(s[:, c*512:(c+1)*512],
                                             ps[0:BC, c*512:(c+1)*512],
                                             bsb[:, c*512:(c+1)*512])
                    for c in range(2):
                        nc.scalar.activation(dst[:, c*512:(c+1)*512],
                                             s[:, c*512:(c+1)*512], Tanh)
                else:
                    for c in range(2):
                        nc.scalar.activation(dst[:, c*512:(c+1)*512],
                                             ps[0:BC, c*512:(c+1)*512], Tanh)

            def z2_post(ps2):
                """tanh + transposes + copy -> new pending z2T tile."""
                z2n = apool.tile([BC, H], DT, tag="z2n")
                tanh_step(ps2, z2n)
                pst2 = pstpool.tile([128, KH * BC], DT, tag="pst")
                for k in range(KH):
                    nc.tensor.transpose(pst2[:, k*BC:(k+1)*BC], z2n[:, k*128:(k+1)*128], id_sb[:])
                z2T_new = spool.tile([128, KH * BC], DT, tag="z2T")
                nc.scalar.activation(z2T_new[:], pst2[:], Copy)
                return z2T_new

            def body():
                nonlocal xts
                xts = {}
                # initial state (zeros, DMA'd so the tiles are typed producers)
                z12T = spool.tile([128, KH * BC], DT, tag="z12T")
                z2T = spool.tile([128, KH * BC], DT, tag="z2T")
                nc.sync.dma_start(z12T[:], ZRO[:])
                nc.sync.dma_start(z2T[:], ZRO[:])
                # prologue: prime XT, open step-0 z1 group, full step-0 z2 group
                for l in range(min(3, L_steps)):
                    fetch_xt(l)
                ps1 = ps1pool.tile([2*BC, H], F32, tag="ps1")
                g1 = groups_for(xts[0], z12T, wi1t_sb, w1t_sb, 0)
                emit_mm(ps1, g1, 0, N_OPEN)
                ps2 = ps2pool.tile([2*BC, H], F32, tag="ps2")
                g2 = groups_for(xts[0], z2T, wi2t_sb, w2t_sb, 1)
                emit_mm(ps2, g2, 0, 99)
                z2T_pending = z2_post(ps2)
                ps2 = g2 = None

                z1n_final = None
                for l in range(L_steps):
                    even = (l % 2 == 0)
                    last = (l == L_steps - 1)
                    fetch_xt(l + 3)

                    # close this step's z1 accumulation (state entering step l)
                    emit_mm(ps1, g1, N_OPEN, 99)

                    # z2 state after step l: updated on even steps
                    if even:
                        z2T = z2T_pending

                    # finish the z2 matmul group for step l+1 (PE fill before tanh wait)
                    if (not last) and (l + 1) % 2 == 0 and ps2 is not None:
                        emit_mm(ps2, g2, N_Z2A, 99)

                    # tanh of this step's z1 (ahead of any z2 ACT work)
                    if last:
                        z1n_f32 = fpool.tile([BC, H], F32)
                        tanh_step(ps1, z1n_f32)
                        z1n_final = z1n_f32
                        break
                    z1n = apool.tile([BC, H], DT, tag="z1n")
                    tanh_step(ps1, z1n)

                    # open next step's z1 group (independent fill before the transposes)
                    ps1 = ps1pool.tile([2*BC, H], F32, tag="ps1")
                    g1 = groups_for(xts[l + 1], z12T, wi1t_sb, w1t_sb, 0)
                    # note: g1 references z12T of step l-1 here only for the X part;
                    # the z entries are re-created below after z12T is updated.
                    emit_mm(ps1, g1, 0, N_OPEN)

                    # transpose z1n
                    pst1 = pstpool.tile([128, KH * BC], DT, tag="pst")
                    for k in range(KH):
                        nc.tensor.transpose(pst1[:, k*BC:(k+1)*BC], z1n[:, k*128:(k+1)*128], id_sb[:])

                    # z2 epilogue for step l+1 (tanh_z2 queues behind tanh_z1 on ACT;
                    # its transposes fill the PE while DVE does the add below)
                    if (not last) and (l + 1) % 2 == 0 and ps2 is not None:
                        z2T_pending = z2_post(ps2)
                        ps2 = g2 = None

                    # z12T = z1nT + z2T(after this step)
                    z12T = spool.tile([128, KH * BC], DT, tag="z12T")
                    for c in range(2):
                        nc.vector.tensor_add(z12T[:, c*256:(c+1)*256], pst1[:, c*256:(c+1)*256], z2T[:, c*256:(c+1)*256])
                    g1 = groups_for(xts[l + 1], z12T, wi1t_sb, w1t_sb, 0)

                    # open the z2 group for step l+2 at the even-step tail
                    # (fills the PE while the add completes)
                    if even and l + 2 < L_steps:
                        ps2 = ps2pool.tile([2*BC, H], F32, tag="ps2")
                        g2 = groups_for(xts[l + 2], z2T, wi2t_sb, w2t_sb, 1)
                        emit_mm(ps2, g2, 0, N_Z2A)

                    if l >= 1:
                        xts.pop(l - 1, None)

                return z1n_final

            if reps > 1:
                with tc.For_i(0, reps, 1):
                    z1n_final = body()
            else:
                z1n_final = body()
            nc.sync.dma_start(OUT[:], z1n_final[:])
    nc.compile()
    return nc


def _get_nc(L_steps, with_bias, reps=1, mode=MODE):
    key = (L_steps, with_bias, reps, mode)
    if key not in _CACHE:
        if isinstance(mode, tuple):
            _CACHE[key] = _build_h(L_steps, with_bias, reps, mode[0], mode[1])
        elif mode in ("v2", "vb2", "v3", "v3m"):
            _CACHE[key] = _build_v2(L_steps, with_bias, reps, mode)
        elif mode in ("h16", "hb16"):
            _CACHE[key] = _build_h(L_steps, with_bias, reps, mode)
        else:
            _CACHE[key] = _build(L_steps, with_bias, reps, mode)
    return _CACHE[key]


def _np_dt(mode):
    if mode in ("hb16", "vb2"):
        import ml_dtypes
        return ml_dtypes.bfloat16
    return np.float16 if mode in ("f16", "h16", "v2", "v3", "v3m") else np.float32


def _prep_in_maps(X, W_in1, b_in1, W_rec1, W_in2, b_in2, W_rec2, L_steps, mode=MODE):
    dt = _np_dt(mode)
    if mode == "v3m":
        import ml_dtypes
        dtw = ml_dtypes.bfloat16
    else:
        dtw = dt
    with_bias = bool(np.any(b_in1) or np.any(b_in2))
    w1t = np.ascontiguousarray(W_rec1.T.astype(dtw))
    w2t = np.ascontiguousarray(W_rec2.T.astype(dtw))
    wi1t = np.ascontiguousarray(W_in1.T.astype(dtw))
    wi2t = np.ascontiguousarray(W_in2.T.astype(dtw))
    if mode in ("h16", "hb16", "v2", "vb2", "v3", "v3m"):
        idn_key = "IDN2"
        idn = np.ascontiguousarray(np.vstack([np.eye(64, dtype=dt)] * 2))
    else:
        idn_key = "IDN"
        idn = np.eye(64, dtype=dt)
    zro = np.zeros((128, KH * BC), dt)
    in_maps = []
    for c in range(NC):
        xt = np.ascontiguousarray(
            X[c*BC:(c+1)*BC, :L_steps, :].transpose(1, 2, 0).astype(dt))
        m = {"XT": xt, "W1T": w1t, "W2T": w2t, "Wi1T": wi1t, "Wi2T": wi2t,
             idn_key: idn, "ZRO": zro}
        if mode in ("h16", "hb16"):
            m["ID128"] = np.eye(128, dtype=dt)
        if with_bias:
            m["BIA"] = np.ascontiguousarray(
                np.stack([b_in1[:, 0], b_in2[:, 0]]).astype(dtw))
            m["ONE"] = np.ones((1, BC), dt)
        in_maps.append(m)
    return in_maps, with_bias


def run_device(X, W_in1, b_in1, W_rec1, W_in2, b_in2, W_rec2, L_steps=L, mode=MODE):
    """Run the recurrence on 8 cores; returns z1_final (B, H) float32."""
    from concourse.bass_utils import run_bass_kernel_spmd
    in_maps, with_bias = _prep_in_maps(X, W_in1, b_in1, W_rec1, W_in2, b_in2,
                                       W_rec2, L_steps, mode)
    nc = _get_nc(L_steps, with_bias, 1, mode)
    res = run_bass_kernel_spmd(nc, in_maps, list(range(NC)))
    return np.concatenate([res.results[c]["OUT"] for c in range(NC)], axis=0)


def kernel(X, W_in1, b_in1, W_rec1, W_in2, b_in2, W_rec2, W_out, b_out):
    X = np.asarray(X); W_out = np.asarray(W_out); b_out = np.asarray(b_out)
    assert X.shape == (B, L, I), f"unexpected X shape {X.shape}"
    z1 = run_device(X, np.asarray(W_in1), np.asarray(b_in1),
                    np.asarray(W_rec1), np.asarray(W_in2), np.asarray(b_in2),
                    np.asarray(W_rec2))
    out = np.tanh(z1.astype(np.float64) @ W_out.astype(np.float64).T
                  + b_out.astype(np.float64)[:, 0])
    return out.reshape(B, 1).astype(np.float32)

